# revision 1
# baseline (speedup 1.0000x reference)
"""LoRA attention kernel for 8 trn2 NeuronCores, tensor-parallel over heads.

Sharding: core s owns heads 2s, 2s+1 (a 128-row slice of the HD=1024 dim).
Each core computes q/k/v projections (base + LoRA fused), attention for its
4 (batch, head) pairs, and a partial output projection; the host sums the 8
partials and adds b_out.

Layouts (per core, on-chip):
  xT   [C=1024, B*N=4096]   activations transposed (contraction dim C on
                            partitions, 8 chunks of 128)
  qT/kT/vT [128, 4096]      2 heads x 64 dims on partitions
  attention runs in S^T layout: S^T[k, q] = K^T.T @ Q^T per 128-key chunk,
  exp via ScalarE (mask folded in as a per-partition additive bias), then
  O^T accumulated with lhsT = [V | ones] so the softmax denominator falls
  out of the same matmuls as PSUM row 64.
"""

import numpy as np

import concourse.bass as bass
import concourse.tile as tile
from concourse import bacc, mybir
from concourse.bass_utils import run_bass_kernel_spmd

H, D, R, C, B, N = 16, 64, 10, 1024, 2, 2048
BN = B * N
SCALING = 1.0 / R
ATT_SCALE = float(D) ** -0.5
NCORES = 8
F32 = mybir.dt.float32
F32R = mybir.dt.float32r
NCH = BN // 512  # 8 n-chunks of 512
CCH = C // 128  # 8 contraction chunks
KCH = N // 128  # 16 key chunks per (b,h)
QCH = N // 512  # 4 query chunks per (b,h)


def build_nc(dbg=False):
    nc = bacc.Bacc("TRN2", target_bir_lowering=False, debug=False,
                   num_devices=NCORES)
    if dbg:
        dbg_q = nc.dram_tensor("dbg_q", [128, BN], F32, kind="ExternalOutput")
        dbg_k = nc.dram_tensor("dbg_k", [128, BN], F32, kind="ExternalOutput")
        dbg_v = nc.dram_tensor("dbg_v", [128, BN], F32, kind="ExternalOutput")
        dbg_ao = nc.dram_tensor("dbg_ao", [128, BN], F32, kind="ExternalOutput")
    xT = nc.dram_tensor("xT", [C, BN], F32R, kind="ExternalInput")
    wqT = nc.dram_tensor("wqT", [C, 128], F32R, kind="ExternalInput")
    wkT = nc.dram_tensor("wkT", [C, 128], F32R, kind="ExternalInput")
    wvT = nc.dram_tensor("wvT", [C, 128], F32R, kind="ExternalInput")
    aT = nc.dram_tensor("aT", [C, 64], F32R, kind="ExternalInput")
    bB = nc.dram_tensor("bB", [42, 256], F32R, kind="ExternalInput")
    bq = nc.dram_tensor("bq", [128, 1], F32, kind="ExternalInput")
    bv = nc.dram_tensor("bv", [128, 1], F32, kind="ExternalInput")
    woT = nc.dram_tensor("woT", [CCH, 128, 128], F32R, kind="ExternalInput")
    idn = nc.dram_tensor("idn", [128, 128], F32R, kind="ExternalInput")
    ones = nc.dram_tensor("ones", [128, KCH], F32R, kind="ExternalInput")
    mb = nc.dram_tensor("mb", [128, B * KCH], F32, kind="ExternalInput")
    yT = nc.dram_tensor("yT", [CCH, 128, BN], F32, kind="ExternalOutput")

    with tile.TileContext(nc) as tc:
        with (
            tc.tile_pool(name="wts", bufs=1) as wts,
            tc.tile_pool(name="acts", bufs=1) as acts,
            tc.tile_pool(name="xin", bufs=3) as xin,
            tc.tile_pool(name="zt", bufs=2) as ztp,
            tc.tile_pool(name="pt", bufs=6) as ptp,
            tc.tile_pool(name="vsb", bufs=2) as vsbp,
            tc.tile_pool(name="rec", bufs=2) as recp,
            tc.tile_pool(name="rbc", bufs=2) as rbcp,
            tc.tile_pool(name="yout", bufs=4) as youtp,
            tc.tile_pool(name="ps_s", bufs=2, space="PSUM") as ps_s,
            tc.tile_pool(name="ps_s2", bufs=2, space="PSUM") as ps_s2,
            tc.tile_pool(name="ps_o", bufs=2, space="PSUM") as ps_o,
        ):
            # --- resident weights ---
            wq_s = wts.tile([128, CCH, 128], F32R)
            nc.sync.dma_start(wq_s[:], wqT.ap().rearrange("(i p) m -> p i m", p=128))
            wk_s = wts.tile([128, CCH, 128], F32R)
            nc.sync.dma_start(wk_s[:], wkT.ap().rearrange("(i p) m -> p i m", p=128))
            wv_s = wts.tile([128, CCH, 128], F32R)
            nc.sync.dma_start(wv_s[:], wvT.ap().rearrange("(i p) m -> p i m", p=128))
            a_s = wts.tile([128, CCH, 64], F32R)
            nc.sync.dma_start(a_s[:], aT.ap().rearrange("(i p) m -> p i m", p=128))
            bB_s = wts.tile([42, 256], F32R)
            nc.sync.dma_start(bB_s[:], bB.ap())
            bq_s = wts.tile([128, 1], F32)
            nc.sync.dma_start(bq_s[:], bq.ap())
            bv_s = wts.tile([128, 1], F32)
            nc.sync.dma_start(bv_s[:], bv.ap())
            wo_s = wts.tile([128, CCH, 128], F32R)
            nc.sync.dma_start(wo_s[:], woT.ap().rearrange("i p m -> p i m"))
            mb_s = wts.tile([128, B * KCH], F32)
            nc.sync.dma_start(mb_s[:], mb.ap())
            ident = wts.tile([128, 128], F32R)
            nc.sync.dma_start(ident[:], idn.ap())
            ones_s = wts.tile([128, KCH], F32R)
            nc.sync.dma_start(ones_s[:], ones.ap())

            # --- persistent activations ---
            qT = acts.tile([128, BN], F32R)
            kT = acts.tile([128, BN], F32R)
            vT = acts.tile([128, BN], F32R)
            aoT = acts.tile([128, BN], F32R)

            xT_r = xT.ap().rearrange("(i p) n -> p i n", p=128)

            # ---------- phase 1: projections ----------
            for nch in range(NCH):
                nsl = bass.ts(nch, 512)
                x_t = xin.tile([128, CCH, 512], F32R)
                nc.sync.dma_start(x_t[:], xT_r[:, :, nsl])

                z_ps = ps_o.tile([64, 512], F32, tag="o")
                for i in range(CCH):
                    nc.tensor.matmul(z_ps[:], (a_s[:, i, :]), (x_t[:, i, :]),
                                     start=(i == 0), stop=(i == CCH - 1))
                z_t = ztp.tile([64, 512], F32R)
                nc.vector.tensor_copy(z_t[:], z_ps[:])

                q_ps = ps_s.tile([128, 512], F32, tag="s")
                for i in range(CCH):
                    nc.tensor.matmul(q_ps[:], (wq_s[:, i, :]), (x_t[:, i, :]),
                                     start=(i == 0), stop=False)
                nc.tensor.matmul(q_ps[:], (bB_s[0:R, 0:128]), (z_t[0:R, :]),
                                 start=False, stop=True)
                nc.scalar.activation(qT[:, nsl], q_ps[:],
                                     mybir.ActivationFunctionType.Identity,
                                     bias=bq_s[:])

                k_ps = ps_s.tile([128, 512], F32, tag="s")
                for i in range(CCH):
                    nc.tensor.matmul(k_ps[:], (wk_s[:, i, :]), (x_t[:, i, :]),
                                     start=(i == 0), stop=(i == CCH - 1))
                nc.vector.tensor_copy(kT[:, nsl], k_ps[:])

                v_ps = ps_s.tile([128, 512], F32, tag="s")
                for i in range(CCH):
                    nc.tensor.matmul(v_ps[:], (wv_s[:, i, :]), (x_t[:, i, :]),
                                     start=(i == 0), stop=False)
                nc.tensor.matmul(v_ps[:], (bB_s[32:32 + R, 128:256]),
                                 (z_t[32:32 + R, :]), start=False, stop=True)
                nc.scalar.activation(vT[:, nsl], v_ps[:],
                                     mybir.ActivationFunctionType.Identity,
                                     bias=bv_s[:])

            # ---------- phase 2: attention ----------
            for b in range(B):
                for hh in range(2):
                    hsl = bass.ds(hh * 64, 64)
                    kb = b * N
                    v_sb = vsbp.tile([128, KCH, 65], F32R)
                    nc.vector.tensor_copy(v_sb[:, :, 64:65], ones_s[:])
                    for g in range(2):
                        vt_ps = ps_s.tile([128, 8, 64], F32R, tag="s")
                        for j in range(8):
                            kc = g * 8 + j
                            nc.tensor.transpose(
                                vt_ps[:, j, :],
                                vT[hsl, bass.ds(kb + kc * 128, 128)],
                                ident[hsl, hsl])
                        nc.vector.tensor_copy(
                            v_sb[:, g * 8:(g + 1) * 8, 0:64], vt_ps[:])

                    for qc in range(QCH):
                        qsl = bass.ds(kb + qc * 512, 512)
                        q_ap = qT[hsl, qsl]
                        o_ps = ps_o.tile([65, 512], F32, tag="o")
                        for g in range(KCH // 2):
                            s_ps = ps_s2.tile([128, 2, 512], F32, tag="s2")
                            for j in range(2):
                                kc = g * 2 + j
                                nc.tensor.matmul(
                                    s_ps[:, j, :],
                                    (kT[hsl, bass.ds(kb + kc * 128, 128)]),
                                    (q_ap), start=True, stop=True)
                            p_sb = ptp.tile([128, 2, 512], F32R)
                            nc.scalar.activation(
                                p_sb[:], s_ps[:],
                                mybir.ActivationFunctionType.Exp,
                                bias=mb_s[:, bass.ds(b * KCH + g * 2, 1)],
                                scale=ATT_SCALE)
                            for j in range(2):
                                kc = g * 2 + j
                                nc.tensor.matmul(o_ps[:], (v_sb[:, kc, :]),
                                                 (p_sb[:, j, :]),
                                                 start=(kc == 0),
                                                 stop=(kc == KCH - 1))
                        rec = recp.tile([1, 512], F32)
                        nc.vector.reciprocal(rec[:], o_ps[64:65, :])
                        rbc = rbcp.tile([64, 512], F32)
                        nc.gpsimd.partition_broadcast(rbc[:], rec[:])
                        nc.vector.tensor_mul(aoT[hsl, qsl], o_ps[0:64, :], rbc[:])

            if dbg:
                nc.sync.dma_start(dbg_q.ap(), qT[:].bitcast(F32))
                nc.sync.dma_start(dbg_k.ap(), kT[:].bitcast(F32))
                nc.sync.dma_start(dbg_v.ap(), vT[:].bitcast(F32))
                nc.sync.dma_start(dbg_ao.ap(), aoT[:].bitcast(F32))

            # ---------- phase 3: output projection ----------
            for nch in range(NCH):
                nsl = bass.ts(nch, 512)
                for ci in range(CCH):
                    y_ps = ps_s.tile([128, 512], F32, tag="s")
                    nc.tensor.matmul(y_ps[:], (wo_s[:, ci, :]), (aoT[:, nsl]),
                                     start=True, stop=True)
                    y_sb = youtp.tile([128, 512], F32)
                    if ci % 2 == 0:
                        nc.scalar.copy(y_sb[:], y_ps[:])
                    else:
                        nc.vector.tensor_copy(y_sb[:], y_ps[:])
                    nc.sync.dma_start(yT.ap()[ci, :, nsl], y_sb[:])
    nc.compile()
    return nc


_NC = None


def _get_nc():
    global _NC
    if _NC is None:
        _NC = build_nc()
    return _NC


def _bB(Bq_sl, Bv_sl):
    out = np.zeros((42, 256), np.float32)
    out[0:R, 0:128] = (Bq_sl * SCALING).T
    out[32:32 + R, 128:256] = (Bv_sl * SCALING).T
    return out


def _prep_in_maps(inputs):
    x = np.asarray(inputs["x"], np.float32)
    mask = np.asarray(inputs["mask"])
    W_qkv = np.asarray(inputs["W_qkv"], np.float32)
    Wq_base = np.asarray(inputs["Wq_base"], np.float32)
    bq = np.asarray(inputs["bq"], np.float32)
    Aq = np.asarray(inputs["Aq"], np.float32)
    Bq = np.asarray(inputs["Bq"], np.float32)
    Wv_base = np.asarray(inputs["Wv_base"], np.float32)
    bv = np.asarray(inputs["bv"], np.float32)
    Av = np.asarray(inputs["Av"], np.float32)
    Bv = np.asarray(inputs["Bv"], np.float32)
    W_out = np.asarray(inputs["W_out"], np.float32)

    xT = np.ascontiguousarray(x.reshape(BN, C).T)
    Wq_eff = W_qkv[0:H * D] + Wq_base
    Wk = W_qkv[H * D:2 * H * D]
    Wv_eff = W_qkv[2 * H * D:3 * H * D] + Wv_base
    aT = np.zeros((C, 64), np.float32)
    aT[:, 0:R] = Aq.T
    aT[:, 32:32 + R] = Av.T
    mbias = np.where(mask.reshape(BN), 0.0, -1e5).astype(np.float32)
    mb = np.ascontiguousarray(mbias.reshape(B * KCH, 128).T)

    in_maps = []
    for s in range(NCORES):
        sl = slice(s * 128, (s + 1) * 128)
        in_maps.append({
            "xT": xT,
            "wqT": np.ascontiguousarray(Wq_eff[sl].T),
            "wkT": np.ascontiguousarray(Wk[sl].T),
            "wvT": np.ascontiguousarray(Wv_eff[sl].T),
            "aT": aT,
            "bB": _bB(Bq[sl], Bv[sl]),
            "bq": np.ascontiguousarray(bq[sl, None]),
            "bv": np.ascontiguousarray(bv[sl, None]),
            "woT": np.ascontiguousarray(
                W_out[:, sl].reshape(CCH, 128, 128).transpose(0, 2, 1)),
            "mb": mb,
            "idn": np.eye(128, dtype=np.float32),
            "ones": np.ones((128, KCH), np.float32),
        })
    return in_maps


def _assemble(results, b_out):
    acc = np.zeros((C, BN), np.float64)
    for r in results:
        acc += r["yT"].reshape(C, BN)
    out = acc.T.astype(np.float32) + np.asarray(b_out, np.float32)[None, :]
    return np.ascontiguousarray(out.reshape(B, N, C))


def kernel(**inputs):
    nc = _get_nc()
    in_maps = _prep_in_maps(inputs)
    res = run_bass_kernel_spmd(nc, in_maps, core_ids=list(range(NCORES)))
    return _assemble(res.results, inputs["b_out"])


def run_traced(inputs):
    """test harness hook: returns (output, exec_time_ns)."""
    nc = _get_nc()
    in_maps = _prep_in_maps(inputs)
    res = run_bass_kernel_spmd(nc, in_maps, core_ids=list(range(NCORES)),
                               trace=True)
    return _assemble(res.results, inputs["b_out"]), res.exec_time_ns



# revision 2
# speedup vs baseline: 6.4282x; 6.4282x over previous
"""LoRA attention kernel for 8 trn2 NeuronCores, tensor-parallel over heads.

Sharding: core s owns heads 2s, 2s+1 (a 128-row slice of the HD=1024 dim).
Host->device traffic is minimized (the axon tunnel moves ~45MB/s, so wire
bytes dominate the spmd-call wall time):
  - x is shipped token-sharded (each core gets 512 of the 4096 tokens,
    transposed to [C, 512]) and AllGathered on-device over NeuronLink.
  - each core computes q/k/v projections (base + LoRA fused), attention for
    its 4 (batch, head) pairs, and a partial output projection [C, BN];
    the partials are ReduceScattered on-device so each core returns only its
    128-row slice of the final y^T. The host stacks the 8 slices and adds
    b_out.

Layouts (per core, on-chip):
  xT   [C=1024, B*N=4096]   activations transposed (contraction dim C on
                            partitions, 8 chunks of 128)
  qT/kT/vT [128, 4096]      2 heads x 64 dims on partitions
  attention runs in S^T layout: S^T[k, q] = K^T.T @ Q^T per 128-key chunk,
  exp via ScalarE (mask folded in as a per-partition additive bias), then
  O^T accumulated with lhsT = [V | ones] so the softmax denominator falls
  out of the same matmuls as PSUM row 64.
"""

import numpy as np

import concourse.bass as bass
import concourse.tile as tile
from concourse import bacc, mybir
from concourse.bass_utils import run_bass_kernel_spmd

H, D, R, C, B, N = 16, 64, 10, 1024, 2, 2048
BN = B * N
SCALING = 1.0 / R
ATT_SCALE = float(D) ** -0.5
NCORES = 8
F32 = mybir.dt.float32
F32R = mybir.dt.float32r
NCH = BN // 512  # 8 n-chunks of 512
CCH = C // 128  # 8 contraction chunks
KCH = N // 128  # 16 key chunks per (b,h)
QCH = N // 512  # 4 query chunks per (b,h)
NSH = BN // NCORES  # 512 tokens per core shard


def build_nc(dbg=False):
    nc = bacc.Bacc("TRN2", target_bir_lowering=False, debug=False,
                   num_devices=NCORES)
    if dbg:
        dbg_q = nc.dram_tensor("dbg_q", [128, BN], F32, kind="ExternalOutput")
        dbg_k = nc.dram_tensor("dbg_k", [128, BN], F32, kind="ExternalOutput")
        dbg_v = nc.dram_tensor("dbg_v", [128, BN], F32, kind="ExternalOutput")
        dbg_ao = nc.dram_tensor("dbg_ao", [128, BN], F32, kind="ExternalOutput")
    xTs = nc.dram_tensor("xTs", [C, NSH], F32R, kind="ExternalInput")
    wqT = nc.dram_tensor("wqT", [C, 128], F32R, kind="ExternalInput")
    wkT = nc.dram_tensor("wkT", [C, 128], F32R, kind="ExternalInput")
    wvT = nc.dram_tensor("wvT", [C, 128], F32R, kind="ExternalInput")
    aT = nc.dram_tensor("aT", [C, 64], F32R, kind="ExternalInput")
    bB = nc.dram_tensor("bB", [42, 256], F32R, kind="ExternalInput")
    bq = nc.dram_tensor("bq", [128, 1], F32, kind="ExternalInput")
    bv = nc.dram_tensor("bv", [128, 1], F32, kind="ExternalInput")
    woT = nc.dram_tensor("woT", [CCH, 128, 128], F32R, kind="ExternalInput")
    idn = nc.dram_tensor("idn", [128, 128], F32R, kind="ExternalInput")
    ones = nc.dram_tensor("ones", [128, KCH], F32R, kind="ExternalInput")
    mb = nc.dram_tensor("mb", [128, B * KCH], F32, kind="ExternalInput")
    yTs = nc.dram_tensor("yTs", [128, BN], F32, kind="ExternalOutput")

    with tile.TileContext(nc) as tc:
        with (
            tc.tile_pool(name="dram", bufs=1, space="DRAM") as dram,
            tc.tile_pool(name="wts", bufs=1) as wts,
            tc.tile_pool(name="acts", bufs=1) as acts,
            tc.tile_pool(name="xin", bufs=3) as xin,
            tc.tile_pool(name="zt", bufs=2) as ztp,
            tc.tile_pool(name="pt", bufs=6) as ptp,
            tc.tile_pool(name="vsb", bufs=2) as vsbp,
            tc.tile_pool(name="rec", bufs=2) as recp,
            tc.tile_pool(name="rbc", bufs=2) as rbcp,
            tc.tile_pool(name="yout", bufs=4) as youtp,
            tc.tile_pool(name="ps_s", bufs=2, space="PSUM") as ps_s,
            tc.tile_pool(name="ps_s2", bufs=2, space="PSUM") as ps_s2,
            tc.tile_pool(name="ps_o", bufs=2, space="PSUM") as ps_o,
        ):
            # --- DRAM bounce buffers for collectives ---
            xs_b = dram.tile([C, NSH], F32R)
            xg_b = dram.tile([NCH, C, NSH], F32R, addr_space="Shared")
            y_b = dram.tile([CCH, 128, BN], F32)
            yr_b = dram.tile([128, BN], F32)

            # gather the full xT across cores: core s contributes tokens
            # [512s, 512s+512), so gathered chunk nch = token chunk nch.
            nc.sync.dma_start(xs_b[:], xTs.ap())
            nc.gpsimd.collective_compute(
                "AllGather", mybir.AluOpType.bypass,
                replica_groups=[list(range(NCORES))],
                ins=[xs_b.opt()], outs=[xg_b.opt()])

            # --- resident weights ---
            wq_s = wts.tile([128, CCH, 128], F32R)
            nc.sync.dma_start(wq_s[:], wqT.ap().rearrange("(i p) m -> p i m", p=128))
            wk_s = wts.tile([128, CCH, 128], F32R)
            nc.sync.dma_start(wk_s[:], wkT.ap().rearrange("(i p) m -> p i m", p=128))
            wv_s = wts.tile([128, CCH, 128], F32R)
            nc.sync.dma_start(wv_s[:], wvT.ap().rearrange("(i p) m -> p i m", p=128))
            a_s = wts.tile([128, CCH, 64], F32R)
            nc.sync.dma_start(a_s[:], aT.ap().rearrange("(i p) m -> p i m", p=128))
            bB_s = wts.tile([42, 256], F32R)
            nc.sync.dma_start(bB_s[:], bB.ap())
            bq_s = wts.tile([128, 1], F32)
            nc.sync.dma_start(bq_s[:], bq.ap())
            bv_s = wts.tile([128, 1], F32)
            nc.sync.dma_start(bv_s[:], bv.ap())
            wo_s = wts.tile([128, CCH, 128], F32R)
            nc.sync.dma_start(wo_s[:], woT.ap().rearrange("i p m -> p i m"))
            mb_s = wts.tile([128, B * KCH], F32)
            nc.sync.dma_start(mb_s[:], mb.ap())
            ident = wts.tile([128, 128], F32R)
            nc.sync.dma_start(ident[:], idn.ap())
            ones_s = wts.tile([128, KCH], F32R)
            nc.sync.dma_start(ones_s[:], ones.ap())

            # --- persistent activations ---
            qT = acts.tile([128, BN], F32R)
            kT = acts.tile([128, BN], F32R)
            vT = acts.tile([128, BN], F32R)
            aoT = acts.tile([128, BN], F32R)

            # ---------- phase 1: projections ----------
            for nch in range(NCH):
                nsl = bass.ts(nch, 512)
                x_t = xin.tile([128, CCH, 512], F32R)
                nc.sync.dma_start(
                    x_t[:], xg_b[nch].rearrange("(i p) m -> p i m", p=128))

                z_ps = ps_o.tile([64, 512], F32, tag="o")
                for i in range(CCH):
                    nc.tensor.matmul(z_ps[:], (a_s[:, i, :]), (x_t[:, i, :]),
                                     start=(i == 0), stop=(i == CCH - 1))
                z_t = ztp.tile([64, 512], F32R)
                nc.vector.tensor_copy(z_t[:], z_ps[:])

                q_ps = ps_s.tile([128, 512], F32, tag="s")
                for i in range(CCH):
                    nc.tensor.matmul(q_ps[:], (wq_s[:, i, :]), (x_t[:, i, :]),
                                     start=(i == 0), stop=False)
                nc.tensor.matmul(q_ps[:], (bB_s[0:R, 0:128]), (z_t[0:R, :]),
                                 start=False, stop=True)
                nc.scalar.activation(qT[:, nsl], q_ps[:],
                                     mybir.ActivationFunctionType.Identity,
                                     bias=bq_s[:])

                k_ps = ps_s.tile([128, 512], F32, tag="s")
                for i in range(CCH):
                    nc.tensor.matmul(k_ps[:], (wk_s[:, i, :]), (x_t[:, i, :]),
                                     start=(i == 0), stop=(i == CCH - 1))
                nc.vector.tensor_copy(kT[:, nsl], k_ps[:])

                v_ps = ps_s.tile([128, 512], F32, tag="s")
                for i in range(CCH):
                    nc.tensor.matmul(v_ps[:], (wv_s[:, i, :]), (x_t[:, i, :]),
                                     start=(i == 0), stop=False)
                nc.tensor.matmul(v_ps[:], (bB_s[32:32 + R, 128:256]),
                                 (z_t[32:32 + R, :]), start=False, stop=True)
                nc.scalar.activation(vT[:, nsl], v_ps[:],
                                     mybir.ActivationFunctionType.Identity,
                                     bias=bv_s[:])

            # ---------- phase 2: attention ----------
            for b in range(B):
                for hh in range(2):
                    hsl = bass.ds(hh * 64, 64)
                    kb = b * N
                    v_sb = vsbp.tile([128, KCH, 65], F32R)
                    nc.vector.tensor_copy(v_sb[:, :, 64:65], ones_s[:])
                    for g in range(2):
                        vt_ps = ps_s.tile([128, 8, 64], F32R, tag="s")
                        for j in range(8):
                            kc = g * 8 + j
                            nc.tensor.transpose(
                                vt_ps[:, j, :],
                                vT[hsl, bass.ds(kb + kc * 128, 128)],
                                ident[hsl, hsl])
                        nc.vector.tensor_copy(
                            v_sb[:, g * 8:(g + 1) * 8, 0:64], vt_ps[:])

                    for qc in range(QCH):
                        qsl = bass.ds(kb + qc * 512, 512)
                        q_ap = qT[hsl, qsl]
                        o_ps = ps_o.tile([65, 512], F32, tag="o")
                        for g in range(KCH // 2):
                            s_ps = ps_s2.tile([128, 2, 512], F32, tag="s2")
                            for j in range(2):
                                kc = g * 2 + j
                                nc.tensor.matmul(
                                    s_ps[:, j, :],
                                    (kT[hsl, bass.ds(kb + kc * 128, 128)]),
                                    (q_ap), start=True, stop=True)
                            p_sb = ptp.tile([128, 2, 512], F32R)
                            nc.scalar.activation(
                                p_sb[:], s_ps[:],
                                mybir.ActivationFunctionType.Exp,
                                bias=mb_s[:, bass.ds(b * KCH + g * 2, 1)],
                                scale=ATT_SCALE)
                            for j in range(2):
                                kc = g * 2 + j
                                nc.tensor.matmul(o_ps[:], (v_sb[:, kc, :]),
                                                 (p_sb[:, j, :]),
                                                 start=(kc == 0),
                                                 stop=(kc == KCH - 1))
                        rec = recp.tile([1, 512], F32)
                        nc.vector.reciprocal(rec[:], o_ps[64:65, :])
                        rbc = rbcp.tile([64, 512], F32)
                        nc.gpsimd.partition_broadcast(rbc[:], rec[:])
                        nc.vector.tensor_mul(aoT[hsl, qsl], o_ps[0:64, :], rbc[:])

            if dbg:
                nc.sync.dma_start(dbg_q.ap(), qT[:].bitcast(F32))
                nc.sync.dma_start(dbg_k.ap(), kT[:].bitcast(F32))
                nc.sync.dma_start(dbg_v.ap(), vT[:].bitcast(F32))
                nc.sync.dma_start(dbg_ao.ap(), aoT[:].bitcast(F32))

            # ---------- phase 3: output projection (partial, per core) ----
            for nch in range(NCH):
                nsl = bass.ts(nch, 512)
                for ci in range(CCH):
                    y_ps = ps_s.tile([128, 512], F32, tag="s")
                    nc.tensor.matmul(y_ps[:], (wo_s[:, ci, :]), (aoT[:, nsl]),
                                     start=True, stop=True)
                    y_sb = youtp.tile([128, 512], F32)
                    if ci % 2 == 0:
                        nc.scalar.copy(y_sb[:], y_ps[:])
                    else:
                        nc.vector.tensor_copy(y_sb[:], y_ps[:])
                    nc.sync.dma_start(y_b[ci, :, nsl], y_sb[:])

            # reduce the partials across cores; core s keeps C-rows slice s
            nc.gpsimd.collective_compute(
                "ReduceScatter", mybir.AluOpType.add,
                replica_groups=[list(range(NCORES))],
                ins=[y_b.opt()], outs=[yr_b.opt()])
            nc.sync.dma_start(yTs.ap(), yr_b[:])
    nc.compile()
    return nc


_NC = None


def _get_nc():
    global _NC
    if _NC is None:
        _NC = build_nc()
    return _NC


def _bB(Bq_sl, Bv_sl):
    out = np.zeros((42, 256), np.float32)
    out[0:R, 0:128] = (Bq_sl * SCALING).T
    out[32:32 + R, 128:256] = (Bv_sl * SCALING).T
    return out


def _prep_in_maps(inputs):
    x = np.asarray(inputs["x"], np.float32)
    mask = np.asarray(inputs["mask"])
    W_qkv = np.asarray(inputs["W_qkv"], np.float32)
    Wq_base = np.asarray(inputs["Wq_base"], np.float32)
    bq = np.asarray(inputs["bq"], np.float32)
    Aq = np.asarray(inputs["Aq"], np.float32)
    Bq = np.asarray(inputs["Bq"], np.float32)
    Wv_base = np.asarray(inputs["Wv_base"], np.float32)
    bv = np.asarray(inputs["bv"], np.float32)
    Av = np.asarray(inputs["Av"], np.float32)
    Bv = np.asarray(inputs["Bv"], np.float32)
    W_out = np.asarray(inputs["W_out"], np.float32)

    xT = np.ascontiguousarray(x.reshape(BN, C).T)
    Wq_eff = W_qkv[0:H * D] + Wq_base
    Wk = W_qkv[H * D:2 * H * D]
    Wv_eff = W_qkv[2 * H * D:3 * H * D] + Wv_base
    aT = np.zeros((C, 64), np.float32)
    aT[:, 0:R] = Aq.T
    aT[:, 32:32 + R] = Av.T
    mbias = np.where(mask.reshape(BN), 0.0, -1e5).astype(np.float32)
    mb = np.ascontiguousarray(mbias.reshape(B * KCH, 128).T)

    in_maps = []
    for s in range(NCORES):
        sl = slice(s * 128, (s + 1) * 128)
        in_maps.append({
            "xTs": np.ascontiguousarray(xT[:, s * NSH:(s + 1) * NSH]),
            "wqT": np.ascontiguousarray(Wq_eff[sl].T),
            "wkT": np.ascontiguousarray(Wk[sl].T),
            "wvT": np.ascontiguousarray(Wv_eff[sl].T),
            "aT": aT,
            "bB": _bB(Bq[sl], Bv[sl]),
            "bq": np.ascontiguousarray(bq[sl, None]),
            "bv": np.ascontiguousarray(bv[sl, None]),
            "woT": np.ascontiguousarray(
                W_out[:, sl].reshape(CCH, 128, 128).transpose(0, 2, 1)),
            "mb": mb,
            "idn": np.eye(128, dtype=np.float32),
            "ones": np.ones((128, KCH), np.float32),
        })
    return in_maps


def _assemble(results, b_out):
    yT = np.concatenate([r["yTs"] for r in results], axis=0)  # [C, BN]
    out = yT.T + np.asarray(b_out, np.float32)[None, :]
    return np.ascontiguousarray(out.reshape(B, N, C).astype(np.float32))


def kernel(**inputs):
    nc = _get_nc()
    in_maps = _prep_in_maps(inputs)
    res = run_bass_kernel_spmd(nc, in_maps, core_ids=list(range(NCORES)))
    return _assemble(res.results, inputs["b_out"])


def run_traced(inputs):
    """test harness hook: returns (output, exec_time_ns)."""
    nc = _get_nc()
    in_maps = _prep_in_maps(inputs)
    res = run_bass_kernel_spmd(nc, in_maps, core_ids=list(range(NCORES)),
                               trace=True)
    return _assemble(res.results, inputs["b_out"]), res.exec_time_ns


# revision 11
# speedup vs baseline: 8.2416x; 1.2821x over previous
"""LoRA attention kernel for 8 trn2 NeuronCores, tensor-parallel over heads.

Sharding: core s owns heads 2s, 2s+1 (a 128-row slice of the HD=1024 dim).
Host->device traffic is minimized (the axon tunnel moves ~45MB/s, so wire
bytes dominate the spmd-call wall time):
  - x is shipped token-sharded (each core gets 512 of the 4096 tokens,
    transposed to [C, 512]) and AllGathered on-device over NeuronLink.
  - x, the large weights, and the output travel the wire as fp16 (adds
    ~5e-4 rel err vs the 2e-2 budget) and are upconverted to f32r on-device
    so all matmul numerics match the f32 version.
  - each core computes q/k/v projections (base + LoRA fused), attention for
    its 4 (batch, head) pairs, and a partial output projection [C, BN];
    the partials are ReduceScattered (f32) on-device so each core returns
    only its 128-row slice of the final y^T. The host stacks the 8 slices
    and adds b_out.

Layouts (per core, on-chip):
  xT   [C=1024, B*N=4096]   activations transposed (contraction dim C on
                            partitions, 8 chunks of 128)
  qT/kT/vT [128, 4096]      2 heads x 64 dims on partitions
  attention runs in S^T layout: S^T[k, q] = K^T.T @ Q^T per 128-key chunk,
  exp via ScalarE (mask folded in as a per-partition additive bias), then
  O^T accumulated with lhsT = [V | ones] so the softmax denominator falls
  out of the same matmuls as PSUM row 64.
"""

import numpy as np

import concourse.bass as bass
import concourse.tile as tile
from concourse import bacc, mybir
from concourse.bass_utils import run_bass_kernel_spmd

H, D, R, C, B, N = 16, 64, 10, 1024, 2, 2048
BN = B * N
SCALING = 1.0 / R
ATT_SCALE = float(D) ** -0.5
NCORES = 8
F32 = mybir.dt.float32
F32R = mybir.dt.float32r
F16 = mybir.dt.float16
NCH = BN // 512  # 8 n-chunks of 512
CCH = C // 128  # 8 contraction chunks
KCH = N // 128  # 16 key chunks per (b,h)
QCH = N // 512  # 4 query chunks per (b,h)
NSH = BN // NCORES  # 512 tokens per core shard


def build_nc(dbg=False):
    nc = bacc.Bacc("TRN2", target_bir_lowering=False, debug=False,
                   num_devices=NCORES)
    if dbg:
        dbg_q = nc.dram_tensor("dbg_q", [128, BN], F32, kind="ExternalOutput")
        dbg_k = nc.dram_tensor("dbg_k", [128, BN], F32, kind="ExternalOutput")
        dbg_v = nc.dram_tensor("dbg_v", [128, BN], F32, kind="ExternalOutput")
        dbg_ao = nc.dram_tensor("dbg_ao", [128, BN], F32, kind="ExternalOutput")
    xTs = nc.dram_tensor("xTs", [C, NSH], F16, kind="ExternalInput")
    wqT = nc.dram_tensor("wqT", [C, 128], F16, kind="ExternalInput")
    wkT = nc.dram_tensor("wkT", [C, 128], F16, kind="ExternalInput")
    wvT = nc.dram_tensor("wvT", [C, 128], F16, kind="ExternalInput")
    aT = nc.dram_tensor("aT", [C, 64], F16, kind="ExternalInput")
    bB = nc.dram_tensor("bB", [42, 256], F32R, kind="ExternalInput")
    bq = nc.dram_tensor("bq", [128, 1], F32, kind="ExternalInput")
    bv = nc.dram_tensor("bv", [128, 1], F32, kind="ExternalInput")
    woT = nc.dram_tensor("woT", [CCH, 128, 128], F16, kind="ExternalInput")
    idn = nc.dram_tensor("idn", [128, 128], F32R, kind="ExternalInput")
    ones = nc.dram_tensor("ones", [128, KCH], F32R, kind="ExternalInput")
    mb = nc.dram_tensor("mb", [128, B * KCH], F32, kind="ExternalInput")
    yTs = nc.dram_tensor("yTs", [128, BN], F16, kind="ExternalOutput")

    with tile.TileContext(nc) as tc:
        with (
            tc.tile_pool(name="dram", bufs=1, space="DRAM") as dram,
            tc.tile_pool(name="wts", bufs=1) as wts,
            tc.tile_pool(name="acts", bufs=1) as acts,
            tc.tile_pool(name="xin", bufs=2) as xin,
            tc.tile_pool(name="xhp", bufs=2) as xhp,
            tc.tile_pool(name="ycv", bufs=2) as ycv,
            tc.tile_pool(name="zt", bufs=2) as ztp,
            tc.tile_pool(name="pt", bufs=6) as ptp,
            tc.tile_pool(name="vsb", bufs=2) as vsbp,
            tc.tile_pool(name="rec", bufs=2) as recp,
            tc.tile_pool(name="rbc", bufs=2) as rbcp,
            tc.tile_pool(name="yout", bufs=4) as youtp,
            tc.tile_pool(name="ps_s", bufs=2, space="PSUM") as ps_s,
            tc.tile_pool(name="ps_s2", bufs=2, space="PSUM") as ps_s2,
            tc.tile_pool(name="ps_o", bufs=2, space="PSUM") as ps_o,
        ):
            # --- DRAM bounce buffers for collectives ---
            xs_b = dram.tile([C, NSH], F16)
            xg_b = dram.tile([NCH, C, NSH], F16, addr_space="Shared")
            y_b = dram.tile([CCH, 128, BN], F32)
            yr_b = dram.tile([128, BN], F32)

            # gather the full xT across cores: core s contributes tokens
            # [512s, 512s+512), so gathered chunk nch = token chunk nch.
            nc.sync.dma_start(xs_b[:], xTs.ap())
            nc.gpsimd.collective_compute(
                "AllGather", mybir.AluOpType.bypass,
                replica_groups=[list(range(NCORES))],
                ins=[xs_b.opt()], outs=[xg_b.opt()])

            # --- resident weights (wire fp16, upconvert to f32r on-chip) ---
            wq_h = wts.tile([128, CCH, 128], F16)
            nc.sync.dma_start(wq_h[:], wqT.ap().rearrange("(i p) m -> p i m", p=128))
            wq_s = wts.tile([128, CCH, 128], F32R)
            nc.gpsimd.tensor_copy(wq_s[:], wq_h[:])
            wk_h = wts.tile([128, CCH, 128], F16)
            nc.sync.dma_start(wk_h[:], wkT.ap().rearrange("(i p) m -> p i m", p=128))
            wk_s = wts.tile([128, CCH, 128], F32R)
            nc.gpsimd.tensor_copy(wk_s[:], wk_h[:])
            wv_h = wts.tile([128, CCH, 128], F16)
            nc.sync.dma_start(wv_h[:], wvT.ap().rearrange("(i p) m -> p i m", p=128))
            wv_s = wts.tile([128, CCH, 128], F32R)
            nc.gpsimd.tensor_copy(wv_s[:], wv_h[:])
            a_h = wts.tile([128, CCH, 64], F16)
            nc.sync.dma_start(a_h[:], aT.ap().rearrange("(i p) m -> p i m", p=128))
            a_s = wts.tile([128, CCH, 64], F32R)
            nc.gpsimd.tensor_copy(a_s[:], a_h[:])
            bB_s = wts.tile([42, 256], F32R)
            nc.sync.dma_start(bB_s[:], bB.ap())
            bq_s = wts.tile([128, 1], F32)
            nc.sync.dma_start(bq_s[:], bq.ap())
            bv_s = wts.tile([128, 1], F32)
            nc.sync.dma_start(bv_s[:], bv.ap())
            wo_h = wts.tile([128, CCH, 128], F16)
            nc.sync.dma_start(wo_h[:], woT.ap().rearrange("i p m -> p i m"))
            wo_s = wts.tile([128, CCH, 128], F32R)
            nc.gpsimd.tensor_copy(wo_s[:], wo_h[:])
            mb_s = wts.tile([128, B * KCH], F32)
            nc.sync.dma_start(mb_s[:], mb.ap())
            ident = wts.tile([128, 128], F32R)
            nc.sync.dma_start(ident[:], idn.ap())
            ones_s = wts.tile([128, KCH], F32R)
            nc.sync.dma_start(ones_s[:], ones.ap())

            # --- persistent activations ---
            qT = acts.tile([128, BN], F32R)
            kT = acts.tile([128, BN], F32R)
            vT = acts.tile([128, BN], F32R)
            aoT = acts.tile([128, BN], F32R)

            # ---------- phase 1: projections ----------
            for nch in range(NCH):
                nsl = bass.ts(nch, 512)
                x_h = xhp.tile([128, CCH, 512], F16)
                nc.sync.dma_start(
                    x_h[:], xg_b[nch].rearrange("(i p) m -> p i m", p=128))
                x_t = xin.tile([128, CCH, 512], F32R)
                nc.gpsimd.tensor_copy(x_t[:], x_h[:])

                z_ps = ps_o.tile([64, 512], F32, tag="o")
                for i in range(CCH):
                    nc.tensor.matmul(z_ps[:], (a_s[:, i, :]), (x_t[:, i, :]),
                                     start=(i == 0), stop=(i == CCH - 1))
                z_t = ztp.tile([64, 512], F32R)
                nc.vector.tensor_copy(z_t[:], z_ps[:])

                q_ps = ps_s.tile([128, 512], F32, tag="s")
                for i in range(CCH):
                    nc.tensor.matmul(q_ps[:], (wq_s[:, i, :]), (x_t[:, i, :]),
                                     start=(i == 0), stop=False)
                nc.tensor.matmul(q_ps[:], (bB_s[0:R, 0:128]), (z_t[0:R, :]),
                                 start=False, stop=True)
                nc.scalar.activation(qT[:, nsl], q_ps[:],
                                     mybir.ActivationFunctionType.Identity,
                                     bias=bq_s[:])

                k_ps = ps_s.tile([128, 512], F32, tag="s")
                for i in range(CCH):
                    nc.tensor.matmul(k_ps[:], (wk_s[:, i, :]), (x_t[:, i, :]),
                                     start=(i == 0), stop=(i == CCH - 1))
                nc.vector.tensor_copy(kT[:, nsl], k_ps[:])

                v_ps = ps_s.tile([128, 512], F32, tag="s")
                for i in range(CCH):
                    nc.tensor.matmul(v_ps[:], (wv_s[:, i, :]), (x_t[:, i, :]),
                                     start=(i == 0), stop=False)
                nc.tensor.matmul(v_ps[:], (bB_s[32:32 + R, 128:256]),
                                 (z_t[32:32 + R, :]), start=False, stop=True)
                nc.scalar.activation(vT[:, nsl], v_ps[:],
                                     mybir.ActivationFunctionType.Identity,
                                     bias=bv_s[:])

            # ---------- phase 2: attention ----------
            for b in range(B):
                for hh in range(2):
                    hsl = bass.ds(hh * 64, 64)
                    kb = b * N
                    v_sb = vsbp.tile([128, KCH, 65], F32R)
                    nc.vector.tensor_copy(v_sb[:, :, 64:65], ones_s[:])
                    for g in range(2):
                        vt_ps = ps_s.tile([128, 8, 64], F32R, tag="s")
                        for j in range(8):
                            kc = g * 8 + j
                            nc.tensor.transpose(
                                vt_ps[:, j, :],
                                vT[hsl, bass.ds(kb + kc * 128, 128)],
                                ident[hsl, hsl])
                        nc.vector.tensor_copy(
                            v_sb[:, g * 8:(g + 1) * 8, 0:64], vt_ps[:])

                    for qc in range(QCH):
                        qsl = bass.ds(kb + qc * 512, 512)
                        q_ap = qT[hsl, qsl]
                        o_ps = ps_o.tile([65, 512], F32, tag="o")
                        for g in range(KCH // 2):
                            s_ps = ps_s2.tile([128, 2, 512], F32, tag="s2")
                            for j in range(2):
                                kc = g * 2 + j
                                nc.tensor.matmul(
                                    s_ps[:, j, :],
                                    (kT[hsl, bass.ds(kb + kc * 128, 128)]),
                                    (q_ap), start=True, stop=True)
                            p_sb = ptp.tile([128, 2, 512], F32R)
                            nc.scalar.activation(
                                p_sb[:], s_ps[:],
                                mybir.ActivationFunctionType.Exp,
                                bias=mb_s[:, bass.ds(b * KCH + g * 2, 1)],
                                scale=ATT_SCALE)
                            for j in range(2):
                                kc = g * 2 + j
                                nc.tensor.matmul(o_ps[:], (v_sb[:, kc, :]),
                                                 (p_sb[:, j, :]),
                                                 start=(kc == 0),
                                                 stop=(kc == KCH - 1))
                        rec = recp.tile([1, 512], F32)
                        nc.vector.reciprocal(rec[:], o_ps[64:65, :])
                        rbc = rbcp.tile([64, 512], F32)
                        nc.gpsimd.partition_broadcast(rbc[:], rec[:])
                        nc.vector.tensor_mul(aoT[hsl, qsl], o_ps[0:64, :], rbc[:])

            if dbg:
                nc.sync.dma_start(dbg_q.ap(), qT[:].bitcast(F32))
                nc.sync.dma_start(dbg_k.ap(), kT[:].bitcast(F32))
                nc.sync.dma_start(dbg_v.ap(), vT[:].bitcast(F32))
                nc.sync.dma_start(dbg_ao.ap(), aoT[:].bitcast(F32))

            # ---------- phase 3: output projection (partial, per core) ----
            for nch in range(NCH):
                nsl = bass.ts(nch, 512)
                for ci in range(CCH):
                    y_ps = ps_s.tile([128, 512], F32, tag="s")
                    nc.tensor.matmul(y_ps[:], (wo_s[:, ci, :]), (aoT[:, nsl]),
                                     start=True, stop=True)
                    y_sb = youtp.tile([128, 512], F32)
                    if ci % 2 == 0:
                        nc.scalar.copy(y_sb[:], y_ps[:])
                    else:
                        nc.vector.tensor_copy(y_sb[:], y_ps[:])
                    nc.sync.dma_start(y_b[ci, :, nsl], y_sb[:])

            # reduce the partials across cores; core s keeps C-rows slice s
            nc.gpsimd.collective_compute(
                "ReduceScatter", mybir.AluOpType.add,
                replica_groups=[list(range(NCORES))],
                ins=[y_b.opt()], outs=[yr_b.opt()])
            # downconvert the final slice to fp16 for the wire
            for ch in range(4):
                csl = bass.ts(ch, 1024)
                y_f = ycv.tile([128, 1024], F32)
                nc.sync.dma_start(y_f[:], yr_b[:, csl])
                y_h = ycv.tile([128, 1024], F16)
                nc.vector.tensor_copy(y_h[:], y_f[:])
                nc.sync.dma_start(yTs.ap()[:, csl], y_h[:])
    nc.compile()
    return nc


_NC = None


def _get_nc():
    global _NC
    if _NC is None:
        _NC = build_nc()
    return _NC


def _bB(Bq_sl, Bv_sl):
    out = np.zeros((42, 256), np.float32)
    out[0:R, 0:128] = (Bq_sl * SCALING).T
    out[32:32 + R, 128:256] = (Bv_sl * SCALING).T
    return out


def _prep_in_maps(inputs):
    x = np.asarray(inputs["x"], np.float32)
    mask = np.asarray(inputs["mask"])
    W_qkv = np.asarray(inputs["W_qkv"], np.float32)
    Wq_base = np.asarray(inputs["Wq_base"], np.float32)
    bq = np.asarray(inputs["bq"], np.float32)
    Aq = np.asarray(inputs["Aq"], np.float32)
    Bq = np.asarray(inputs["Bq"], np.float32)
    Wv_base = np.asarray(inputs["Wv_base"], np.float32)
    bv = np.asarray(inputs["bv"], np.float32)
    Av = np.asarray(inputs["Av"], np.float32)
    Bv = np.asarray(inputs["Bv"], np.float32)
    W_out = np.asarray(inputs["W_out"], np.float32)

    xT = np.ascontiguousarray(x.reshape(BN, C).T)
    Wq_eff = W_qkv[0:H * D] + Wq_base
    Wk = W_qkv[H * D:2 * H * D]
    Wv_eff = W_qkv[2 * H * D:3 * H * D] + Wv_base
    aT = np.zeros((C, 64), np.float32)
    aT[:, 0:R] = Aq.T
    aT[:, 32:32 + R] = Av.T
    mbias = np.where(mask.reshape(BN), 0.0, -1e5).astype(np.float32)
    mb = np.ascontiguousarray(mbias.reshape(B * KCH, 128).T)

    in_maps = []
    for s in range(NCORES):
        sl = slice(s * 128, (s + 1) * 128)
        in_maps.append({
            "xTs": np.ascontiguousarray(
                xT[:, s * NSH:(s + 1) * NSH]).astype(np.float16),
            "wqT": np.ascontiguousarray(Wq_eff[sl].T).astype(np.float16),
            "wkT": np.ascontiguousarray(Wk[sl].T).astype(np.float16),
            "wvT": np.ascontiguousarray(Wv_eff[sl].T).astype(np.float16),
            "aT": aT.astype(np.float16),
            "bB": _bB(Bq[sl], Bv[sl]),
            "bq": np.ascontiguousarray(bq[sl, None]),
            "bv": np.ascontiguousarray(bv[sl, None]),
            "woT": np.ascontiguousarray(
                W_out[:, sl].reshape(CCH, 128, 128).transpose(0, 2, 1)
            ).astype(np.float16),
            "mb": mb,
            "idn": np.eye(128, dtype=np.float32),
            "ones": np.ones((128, KCH), np.float32),
        })
    return in_maps


def _assemble(results, b_out):
    yT = np.concatenate(
        [r["yTs"].astype(np.float32) for r in results], axis=0)  # [C, BN]
    out = yT.T + np.asarray(b_out, np.float32)[None, :]
    return np.ascontiguousarray(out.reshape(B, N, C).astype(np.float32))


def kernel(**inputs):
    nc = _get_nc()
    in_maps = _prep_in_maps(inputs)
    res = run_bass_kernel_spmd(nc, in_maps, core_ids=list(range(NCORES)))
    return _assemble(res.results, inputs["b_out"])


def run_traced(inputs):
    """test harness hook: returns (output, exec_time_ns)."""
    nc = _get_nc()
    in_maps = _prep_in_maps(inputs)
    res = run_bass_kernel_spmd(nc, in_maps, core_ids=list(range(NCORES)),
                               trace=True)
    return _assemble(res.results, inputs["b_out"]), res.exec_time_ns


# revision 19
# speedup vs baseline: 11.1849x; 1.3571x over previous
"""LoRA attention kernel for 8 trn2 NeuronCores, tensor-parallel over heads.

Sharding: core s owns heads 2s, 2s+1 (a 128-row slice of the HD=1024 dim).
Host->device traffic is minimized (the axon tunnel moves ~45MB/s, so wire
bytes dominate the spmd-call wall time):
  - x is shipped token-sharded (each core gets 512 of the 4096 tokens,
    transposed to [C, 512]) and AllGathered on-device over NeuronLink.
  - x, the large weights, and the output travel the wire as fp16 (adds
    ~5e-4 rel err vs the 2e-2 budget) and are upconverted to f32r on-device
    so all matmul numerics match the f32 version.
  - each core computes q/k/v projections (base + LoRA fused), attention for
    its 4 (batch, head) pairs, and a partial output projection [C, BN];
    the partials are ReduceScattered (f32) on-device so each core returns
    only its 128-row slice of the final y^T. The host stacks the 8 slices
    and adds b_out.

Layouts (per core, on-chip):
  xT   [C=1024, B*N=4096]   activations transposed (contraction dim C on
                            partitions, 8 chunks of 128)
  qT/kT/vT [128, 4096]      2 heads x 64 dims on partitions
  attention runs in S^T layout: S^T[k, q] = K^T.T @ Q^T per 128-key chunk,
  exp via ScalarE (mask folded in as a per-partition additive bias), then
  O^T accumulated with lhsT = [V | ones] so the softmax denominator falls
  out of the same matmuls as PSUM row 64.
"""

import numpy as np

import concourse.bass as bass
import concourse.tile as tile
from concourse import bacc, mybir
from concourse.bass_utils import run_bass_kernel_spmd

H, D, R, C, B, N = 16, 64, 10, 1024, 2, 2048
BN = B * N
SCALING = 1.0 / R
ATT_SCALE = float(D) ** -0.5
NCORES = 8
F32 = mybir.dt.float32
F32R = mybir.dt.float32r
F16 = mybir.dt.float16
NCH = BN // 512  # 8 n-chunks of 512
CCH = C // 128  # 8 contraction chunks
KCH = N // 128  # 16 key chunks per (b,h)
QCH = N // 512  # 4 query chunks per (b,h)
NSH = BN // NCORES  # 512 tokens per core shard


def build_nc(dbg=False):
    nc = bacc.Bacc("TRN2", target_bir_lowering=False, debug=False,
                   num_devices=NCORES)
    if dbg:
        dbg_q = nc.dram_tensor("dbg_q", [128, BN], F32, kind="ExternalOutput")
        dbg_k = nc.dram_tensor("dbg_k", [128, BN], F32, kind="ExternalOutput")
        dbg_v = nc.dram_tensor("dbg_v", [128, BN], F32, kind="ExternalOutput")
        dbg_ao = nc.dram_tensor("dbg_ao", [128, BN], F32, kind="ExternalOutput")
    xTs = nc.dram_tensor("xTs", [C, NSH], F16, kind="ExternalInput")
    wqT = nc.dram_tensor("wqT", [C, 128], F16, kind="ExternalInput")
    wkT = nc.dram_tensor("wkT", [C, 128], F16, kind="ExternalInput")
    wvT = nc.dram_tensor("wvT", [C, 128], F16, kind="ExternalInput")
    aT = nc.dram_tensor("aT", [C, 64], F16, kind="ExternalInput")
    bB = nc.dram_tensor("bB", [42, 256], F32R, kind="ExternalInput")
    bq = nc.dram_tensor("bq", [128, 1], F32, kind="ExternalInput")
    bv = nc.dram_tensor("bv", [128, 1], F32, kind="ExternalInput")
    woT = nc.dram_tensor("woT", [CCH, 128, 128], F16, kind="ExternalInput")
    mb = nc.dram_tensor("mb", [128, B * KCH], F32, kind="ExternalInput")
    yq8 = nc.dram_tensor("yq8", [128, BN], mybir.dt.int8, kind="ExternalOutput")
    ysc = nc.dram_tensor("ysc", [128, 1], F32, kind="ExternalOutput")

    with tile.TileContext(nc) as tc:
        with (
            tc.tile_pool(name="dram", bufs=1, space="DRAM") as dram,
            tc.tile_pool(name="wts", bufs=1) as wts,
            tc.tile_pool(name="acts", bufs=1) as acts,
            tc.tile_pool(name="xin", bufs=2) as xin,
            tc.tile_pool(name="xhp", bufs=2) as xhp,
            tc.tile_pool(name="ycv", bufs=1) as ycv,
            tc.tile_pool(name="zt", bufs=2) as ztp,
            tc.tile_pool(name="pt", bufs=4) as ptp,
            tc.tile_pool(name="vsb", bufs=2) as vsbp,
            tc.tile_pool(name="rec", bufs=2) as recp,
            tc.tile_pool(name="rbc", bufs=2) as rbcp,
            tc.tile_pool(name="yout", bufs=4) as youtp,
            tc.tile_pool(name="ps_s", bufs=2, space="PSUM") as ps_s,
            tc.tile_pool(name="ps_s2", bufs=2, space="PSUM") as ps_s2,
            tc.tile_pool(name="ps_o", bufs=2, space="PSUM") as ps_o,
        ):
            # --- DRAM bounce buffers for collectives ---
            xs_b = dram.tile([C, NSH], F16)
            xg_b = dram.tile([NCH, C, NSH], F16, addr_space="Shared")
            y_b = dram.tile([CCH, 128, BN], F32)
            yr_b = dram.tile([128, BN], F32)

            # gather the full xT across cores: core s contributes tokens
            # [512s, 512s+512), so gathered chunk nch = token chunk nch.
            nc.sync.dma_start(xs_b[:], xTs.ap())
            nc.gpsimd.collective_compute(
                "AllGather", mybir.AluOpType.bypass,
                replica_groups=[list(range(NCORES))],
                ins=[xs_b.opt()], outs=[xg_b.opt()])

            # --- resident weights (wire fp16, upconvert to f32r on-chip) ---
            wq_h = wts.tile([128, CCH, 128], F16)
            nc.sync.dma_start(wq_h[:], wqT.ap().rearrange("(i p) m -> p i m", p=128))
            wq_s = wts.tile([128, CCH, 128], F32R)
            nc.gpsimd.tensor_copy(wq_s[:], wq_h[:])
            wk_h = wts.tile([128, CCH, 128], F16)
            nc.sync.dma_start(wk_h[:], wkT.ap().rearrange("(i p) m -> p i m", p=128))
            wk_s = wts.tile([128, CCH, 128], F32R)
            nc.gpsimd.tensor_copy(wk_s[:], wk_h[:])
            wv_h = wts.tile([128, CCH, 128], F16)
            nc.sync.dma_start(wv_h[:], wvT.ap().rearrange("(i p) m -> p i m", p=128))
            wv_s = wts.tile([128, CCH, 128], F32R)
            nc.gpsimd.tensor_copy(wv_s[:], wv_h[:])
            a_h = wts.tile([128, CCH, 64], F16)
            nc.sync.dma_start(a_h[:], aT.ap().rearrange("(i p) m -> p i m", p=128))
            a_s = wts.tile([128, CCH, 64], F32R)
            nc.gpsimd.tensor_copy(a_s[:], a_h[:])
            bB_s = wts.tile([42, 256], F32R)
            nc.sync.dma_start(bB_s[:], bB.ap())
            bq_s = wts.tile([128, 1], F32)
            nc.sync.dma_start(bq_s[:], bq.ap())
            bv_s = wts.tile([128, 1], F32)
            nc.sync.dma_start(bv_s[:], bv.ap())
            wo_h = wts.tile([128, CCH, 128], F16)
            nc.sync.dma_start(wo_h[:], woT.ap().rearrange("i p m -> p i m"))
            wo_s = wts.tile([128, CCH, 128], F32R)
            nc.gpsimd.tensor_copy(wo_s[:], wo_h[:])
            mb_s = wts.tile([128, B * KCH], F32)
            nc.sync.dma_start(mb_s[:], mb.ap())
            # identity for PE transposes, built on-chip: free_idx - part_idx == 0
            io32 = wts.tile([128, 128], mybir.dt.int32)
            nc.gpsimd.iota(io32[:], pattern=[[1, 128]], base=0,
                           channel_multiplier=-1)
            ident = wts.tile([128, 128], F32R)
            nc.gpsimd.tensor_scalar(ident[:], io32[:], 0, None,
                                    mybir.AluOpType.is_equal)
            ones_s = wts.tile([128, KCH], F32R)
            nc.gpsimd.tensor_scalar(ones_s[:], io32[:, 0:KCH], -(1 << 30),
                                    None, mybir.AluOpType.is_gt)

            # --- persistent activations ---
            qT = acts.tile([128, BN], F32R)
            kT = acts.tile([128, BN], F32R)
            vT = acts.tile([128, BN], F32R)
            aoT = acts.tile([128, BN], F32R)

            # ---------- phase 1: projections ----------
            for nch in range(NCH):
                nsl = bass.ts(nch, 512)
                x_h = xhp.tile([128, CCH, 512], F16)
                nc.sync.dma_start(
                    x_h[:], xg_b[nch].rearrange("(i p) m -> p i m", p=128))
                x_t = xin.tile([128, CCH, 512], F32R)
                nc.gpsimd.tensor_copy(x_t[:], x_h[:])

                z_ps = ps_o.tile([64, 512], F32, tag="o")
                for i in range(CCH):
                    nc.tensor.matmul(z_ps[:], (a_s[:, i, :]), (x_t[:, i, :]),
                                     start=(i == 0), stop=(i == CCH - 1))
                z_t = ztp.tile([64, 512], F32R)
                nc.vector.tensor_copy(z_t[:], z_ps[:])

                q_ps = ps_s.tile([128, 512], F32, tag="s")
                for i in range(CCH):
                    nc.tensor.matmul(q_ps[:], (wq_s[:, i, :]), (x_t[:, i, :]),
                                     start=(i == 0), stop=False)
                nc.tensor.matmul(q_ps[:], (bB_s[0:R, 0:128]), (z_t[0:R, :]),
                                 start=False, stop=True)
                nc.scalar.activation(qT[:, nsl], q_ps[:],
                                     mybir.ActivationFunctionType.Identity,
                                     bias=bq_s[:])

                k_ps = ps_s.tile([128, 512], F32, tag="s")
                for i in range(CCH):
                    nc.tensor.matmul(k_ps[:], (wk_s[:, i, :]), (x_t[:, i, :]),
                                     start=(i == 0), stop=(i == CCH - 1))
                nc.vector.tensor_copy(kT[:, nsl], k_ps[:])

                v_ps = ps_s.tile([128, 512], F32, tag="s")
                for i in range(CCH):
                    nc.tensor.matmul(v_ps[:], (wv_s[:, i, :]), (x_t[:, i, :]),
                                     start=(i == 0), stop=False)
                nc.tensor.matmul(v_ps[:], (bB_s[32:32 + R, 128:256]),
                                 (z_t[32:32 + R, :]), start=False, stop=True)
                nc.scalar.activation(vT[:, nsl], v_ps[:],
                                     mybir.ActivationFunctionType.Identity,
                                     bias=bv_s[:])

            # ---------- phase 2: attention ----------
            for b in range(B):
                for hh in range(2):
                    hsl = bass.ds(hh * 64, 64)
                    kb = b * N
                    v_sb = vsbp.tile([128, KCH, 65], F32R)
                    nc.vector.tensor_copy(v_sb[:, :, 64:65], ones_s[:])
                    for g in range(2):
                        vt_ps = ps_s.tile([128, 8, 64], F32R, tag="s")
                        for j in range(8):
                            kc = g * 8 + j
                            nc.tensor.transpose(
                                vt_ps[:, j, :],
                                vT[hsl, bass.ds(kb + kc * 128, 128)],
                                ident[hsl, hsl])
                        nc.vector.tensor_copy(
                            v_sb[:, g * 8:(g + 1) * 8, 0:64], vt_ps[:])

                    for qc in range(QCH):
                        qsl = bass.ds(kb + qc * 512, 512)
                        q_ap = qT[hsl, qsl]
                        o_ps = ps_o.tile([65, 512], F32, tag="o")
                        for g in range(KCH // 2):
                            s_ps = ps_s2.tile([128, 2, 512], F32, tag="s2")
                            for j in range(2):
                                kc = g * 2 + j
                                nc.tensor.matmul(
                                    s_ps[:, j, :],
                                    (kT[hsl, bass.ds(kb + kc * 128, 128)]),
                                    (q_ap), start=True, stop=True)
                            p_sb = ptp.tile([128, 2, 512], F32R)
                            nc.scalar.activation(
                                p_sb[:], s_ps[:],
                                mybir.ActivationFunctionType.Exp,
                                bias=mb_s[:, bass.ds(b * KCH + g * 2, 1)],
                                scale=ATT_SCALE)
                            for j in range(2):
                                kc = g * 2 + j
                                nc.tensor.matmul(o_ps[:], (v_sb[:, kc, :]),
                                                 (p_sb[:, j, :]),
                                                 start=(kc == 0),
                                                 stop=(kc == KCH - 1))
                        rec = recp.tile([1, 512], F32)
                        nc.vector.reciprocal(rec[:], o_ps[64:65, :])
                        rbc = rbcp.tile([64, 512], F32)
                        nc.gpsimd.partition_broadcast(rbc[:], rec[:])
                        nc.vector.tensor_mul(aoT[hsl, qsl], o_ps[0:64, :], rbc[:])

            if dbg:
                nc.sync.dma_start(dbg_q.ap(), qT[:].bitcast(F32))
                nc.sync.dma_start(dbg_k.ap(), kT[:].bitcast(F32))
                nc.sync.dma_start(dbg_v.ap(), vT[:].bitcast(F32))
                nc.sync.dma_start(dbg_ao.ap(), aoT[:].bitcast(F32))

            # ---------- phase 3: output projection (partial, per core) ----
            for nch in range(NCH):
                nsl = bass.ts(nch, 512)
                for ci in range(CCH):
                    y_ps = ps_s.tile([128, 512], F32, tag="s")
                    nc.tensor.matmul(y_ps[:], (wo_s[:, ci, :]), (aoT[:, nsl]),
                                     start=True, stop=True)
                    y_sb = youtp.tile([128, 512], F32)
                    if ci % 2 == 0:
                        nc.scalar.copy(y_sb[:], y_ps[:])
                    else:
                        nc.vector.tensor_copy(y_sb[:], y_ps[:])
                    nc.sync.dma_start(y_b[ci, :, nsl], y_sb[:])

            # reduce the partials across cores; core s keeps C-rows slice s
            nc.gpsimd.collective_compute(
                "ReduceScatter", mybir.AluOpType.add,
                replica_groups=[list(range(NCORES))],
                ins=[y_b.opt()], outs=[yr_b.opt()])
            # int8-quantize the final slice (per-partition scale) for the wire
            y_f = ycv.tile([128, BN], F32)
            nc.sync.dma_start(y_f[:], yr_b[:])
            amax = ycv.tile([128, 1], F32)
            nc.vector.tensor_reduce(amax[:], y_f[:], mybir.AxisListType.X,
                                    mybir.AluOpType.max,
                                    apply_absolute_value=True)
            nc.vector.tensor_scalar_max(amax[:], amax[:], 1e-20)
            nc.sync.dma_start(ysc.ap(), amax[:])
            rcp = ycv.tile([128, 1], F32)
            nc.vector.reciprocal(rcp[:], amax[:])
            qsc = ycv.tile([128, 1], F32)
            nc.vector.tensor_scalar_mul(qsc[:], rcp[:], 127.0)
            yq = ycv.tile([128, BN], mybir.dt.int8)
            nc.vector.tensor_scalar_mul(yq[:], y_f[:], qsc[:])
            nc.sync.dma_start(yq8.ap(), yq[:])
    nc.compile()
    return nc


_NC = None


def _get_nc():
    global _NC
    if _NC is None:
        _NC = build_nc()
    return _NC


def _bB(Bq_sl, Bv_sl):
    out = np.zeros((42, 256), np.float32)
    out[0:R, 0:128] = (Bq_sl * SCALING).T
    out[32:32 + R, 128:256] = (Bv_sl * SCALING).T
    return out


def _prep_in_maps(inputs):
    x = np.asarray(inputs["x"], np.float32)
    mask = np.asarray(inputs["mask"])
    W_qkv = np.asarray(inputs["W_qkv"], np.float32)
    Wq_base = np.asarray(inputs["Wq_base"], np.float32)
    bq = np.asarray(inputs["bq"], np.float32)
    Aq = np.asarray(inputs["Aq"], np.float32)
    Bq = np.asarray(inputs["Bq"], np.float32)
    Wv_base = np.asarray(inputs["Wv_base"], np.float32)
    bv = np.asarray(inputs["bv"], np.float32)
    Av = np.asarray(inputs["Av"], np.float32)
    Bv = np.asarray(inputs["Bv"], np.float32)
    W_out = np.asarray(inputs["W_out"], np.float32)

    xT = np.ascontiguousarray(x.reshape(BN, C).T)
    Wq_eff = W_qkv[0:H * D] + Wq_base
    Wk = W_qkv[H * D:2 * H * D]
    Wv_eff = W_qkv[2 * H * D:3 * H * D] + Wv_base
    aT = np.zeros((C, 64), np.float32)
    aT[:, 0:R] = Aq.T
    aT[:, 32:32 + R] = Av.T
    mbias = np.where(mask.reshape(BN), 0.0, -1e5).astype(np.float32)
    mb = np.ascontiguousarray(mbias.reshape(B * KCH, 128).T)

    in_maps = []
    for s in range(NCORES):
        sl = slice(s * 128, (s + 1) * 128)
        in_maps.append({
            "xTs": np.ascontiguousarray(
                xT[:, s * NSH:(s + 1) * NSH]).astype(np.float16),
            "wqT": np.ascontiguousarray(Wq_eff[sl].T).astype(np.float16),
            "wkT": np.ascontiguousarray(Wk[sl].T).astype(np.float16),
            "wvT": np.ascontiguousarray(Wv_eff[sl].T).astype(np.float16),
            "aT": aT.astype(np.float16),
            "bB": _bB(Bq[sl], Bv[sl]),
            "bq": np.ascontiguousarray(bq[sl, None]),
            "bv": np.ascontiguousarray(bv[sl, None]),
            "woT": np.ascontiguousarray(
                W_out[:, sl].reshape(CCH, 128, 128).transpose(0, 2, 1)
            ).astype(np.float16),
            "mb": mb,
        })
    return in_maps


def _assemble(results, b_out):
    yT = np.concatenate(
        [r["yq8"].astype(np.float32) * (r["ysc"] / 127.0) for r in results],
        axis=0)  # [C, BN]
    out = yT.T + np.asarray(b_out, np.float32)[None, :]
    return np.ascontiguousarray(out.reshape(B, N, C).astype(np.float32))


def kernel(**inputs):
    nc = _get_nc()
    in_maps = _prep_in_maps(inputs)
    res = run_bass_kernel_spmd(nc, in_maps, core_ids=list(range(NCORES)))
    return _assemble(res.results, inputs["b_out"])


def run_traced(inputs):
    """test harness hook: returns (output, exec_time_ns)."""
    nc = _get_nc()
    in_maps = _prep_in_maps(inputs)
    res = run_bass_kernel_spmd(nc, in_maps, core_ids=list(range(NCORES)),
                               trace=True)
    return _assemble(res.results, inputs["b_out"]), res.exec_time_ns


# revision 28
# speedup vs baseline: 15.0719x; 1.3475x over previous
"""LoRA attention kernel for 8 trn2 NeuronCores, tensor-parallel over heads.

Sharding: core s owns heads 2s, 2s+1 (a 128-row slice of the HD=1024 dim).
Host->device traffic is minimized (the axon tunnel moves ~45MB/s, so wire
bytes dominate the spmd-call wall time):
  - x is shipped token-sharded (each core gets 512 of the 4096 tokens,
    transposed to [C, 512]) and AllGathered on-device over NeuronLink.
  - x and the large weights travel the wire as fp16 (adds ~5e-4 rel err vs
    the 2e-2 budget) and are upconverted to f32r on-device so all matmul
    numerics match the f32 version. The per-core slice of the LoRA A matrix
    rides the same AllGather as x (8 extra packed columns).
  - each core computes q/k/v projections (base + LoRA fused), attention for
    its 4 (batch, head) pairs, and a partial output projection [C, BN];
    the partials are ReduceScattered (f32) on-device so each core returns
    only its 128-row slice of the final y^T, int8-quantized with a
    per-row scale (adds ~3e-3 rel err). The host dequantizes, stacks the 8
    slices, and adds b_out.

Layouts (per core, on-chip):
  xT   [C=1024, B*N=4096]   activations transposed (contraction dim C on
                            partitions, 8 chunks of 128)
  qT/kT/vT [128, 4096]      2 heads x 64 dims on partitions
  attention runs in S^T layout: S^T[k, q] = K^T.T @ Q^T per 128-key chunk,
  exp via ScalarE (mask folded in as a per-partition additive bias), then
  O^T accumulated with lhsT = [V | ones] so the softmax denominator falls
  out of the same matmuls as PSUM row 64.
"""

import numpy as np

import jax

try:
    # Each run_bass_kernel_spmd call re-jits (fresh closures inside the
    # library), so the persistent compile cache saves ~80ms/call.
    jax.config.update("jax_compilation_cache_dir", "/tmp/jax_comp_cache")
    jax.config.update("jax_persistent_cache_min_entry_size_bytes", 0)
    jax.config.update("jax_persistent_cache_min_compile_time_secs", 0.0)
except Exception:
    pass

import concourse.bass as bass
import concourse.tile as tile
from concourse import bacc, mybir
from concourse.bass_utils import run_bass_kernel_spmd

H, D, R, C, B, N = 16, 64, 10, 1024, 2, 2048
BN = B * N
SCALING = 1.0 / R
ATT_SCALE = float(D) ** -0.5
NCORES = 8
F32 = mybir.dt.float32
F32R = mybir.dt.float32r
F16 = mybir.dt.float16
NCH = BN // 512  # 8 n-chunks of 512
CCH = C // 128  # 8 contraction chunks
KCH = N // 128  # 16 key chunks per (b,h)
QCH = N // 512  # 4 query chunks per (b,h)
NSH = BN // NCORES  # 512 tokens per core shard


def build_nc(dbg=False):
    nc = bacc.Bacc("TRN2", target_bir_lowering=False, debug=False,
                   num_devices=NCORES)
    if dbg:
        dbg_q = nc.dram_tensor("dbg_q", [128, BN], F32, kind="ExternalOutput")
        dbg_k = nc.dram_tensor("dbg_k", [128, BN], F32, kind="ExternalOutput")
        dbg_v = nc.dram_tensor("dbg_v", [128, BN], F32, kind="ExternalOutput")
        dbg_ao = nc.dram_tensor("dbg_ao", [128, BN], F32, kind="ExternalOutput")
    # xTs carries this core's 512-token slice of x^T plus, in 8 extra
    # columns, this core's 128-row slice of aT (the LoRA A matrices) packed
    # [128,64]->[1024,8] so it rides the same AllGather.
    xTs = nc.dram_tensor("xTs", [C, NSH + 8], F16, kind="ExternalInput")
    wqT = nc.dram_tensor("wqT", [C, 128], F16, kind="ExternalInput")
    wkT = nc.dram_tensor("wkT", [C, 128], F16, kind="ExternalInput")
    wvT = nc.dram_tensor("wvT", [C, 128], F16, kind="ExternalInput")
    bB = nc.dram_tensor("bB", [42, 256], F16, kind="ExternalInput")
    bq = nc.dram_tensor("bq", [128, 1], F32, kind="ExternalInput")
    bv = nc.dram_tensor("bv", [128, 1], F32, kind="ExternalInput")
    woT = nc.dram_tensor("woT", [CCH, 128, 128], F16, kind="ExternalInput")
    mb = nc.dram_tensor("mb", [128, B * KCH], F32, kind="ExternalInput")
    yq8 = nc.dram_tensor("yq8", [128, BN], mybir.dt.int8, kind="ExternalOutput")
    ysc = nc.dram_tensor("ysc", [128, 1], F32, kind="ExternalOutput")

    with tile.TileContext(nc) as tc:
        with (
            tc.tile_pool(name="dram", bufs=1, space="DRAM") as dram,
            tc.tile_pool(name="wts", bufs=1) as wts,
            tc.tile_pool(name="acts", bufs=1) as acts,
            tc.tile_pool(name="xin", bufs=2) as xin,
            tc.tile_pool(name="xhp", bufs=2) as xhp,
            tc.tile_pool(name="ycv", bufs=1) as ycv,
            tc.tile_pool(name="zt", bufs=2) as ztp,
            tc.tile_pool(name="pt", bufs=4) as ptp,
            tc.tile_pool(name="vsb", bufs=2) as vsbp,
            tc.tile_pool(name="rec", bufs=2) as recp,
            tc.tile_pool(name="rbc", bufs=2) as rbcp,
            tc.tile_pool(name="yout", bufs=4) as youtp,
            tc.tile_pool(name="ps_s", bufs=2, space="PSUM") as ps_s,
            tc.tile_pool(name="ps_s2", bufs=2, space="PSUM") as ps_s2,
            tc.tile_pool(name="ps_o", bufs=2, space="PSUM") as ps_o,
        ):
            # --- DRAM bounce buffers for collectives ---
            xs_b = dram.tile([C, NSH + 8], F16)
            xg_b = dram.tile([NCH, C, NSH + 8], F16, addr_space="Shared")
            y_b = dram.tile([CCH, 128, BN], F32)
            yr_b = dram.tile([128, BN], F32)

            # gather the full xT across cores: core s contributes tokens
            # [512s, 512s+512), so gathered chunk nch = token chunk nch.
            nc.sync.dma_start(xs_b[:], xTs.ap())
            nc.gpsimd.collective_compute(
                "AllGather", mybir.AluOpType.bypass,
                replica_groups=[list(range(NCORES))],
                ins=[xs_b.opt()], outs=[xg_b.opt()])

            # --- resident weights (wire fp16, upconvert to f32r on-chip) ---
            wq_h = wts.tile([128, CCH, 128], F16)
            nc.sync.dma_start(wq_h[:], wqT.ap().rearrange("(i p) m -> p i m", p=128))
            wq_s = wts.tile([128, CCH, 128], F32R)
            nc.gpsimd.tensor_copy(wq_s[:], wq_h[:])
            wk_h = wts.tile([128, CCH, 128], F16)
            nc.sync.dma_start(wk_h[:], wkT.ap().rearrange("(i p) m -> p i m", p=128))
            wk_s = wts.tile([128, CCH, 128], F32R)
            nc.gpsimd.tensor_copy(wk_s[:], wk_h[:])
            wv_h = wts.tile([128, CCH, 128], F16)
            nc.sync.dma_start(wv_h[:], wvT.ap().rearrange("(i p) m -> p i m", p=128))
            wv_s = wts.tile([128, CCH, 128], F32R)
            nc.gpsimd.tensor_copy(wv_s[:], wv_h[:])
            # aT arrives inside the gathered x buffer: chunk i's last 8
            # columns unpack to aT rows [128i, 128i+128).
            a_h = wts.tile([128, CCH, 64], F16)
            for i in range(CCH):
                nc.sync.dma_start(
                    a_h[:, i, :].rearrange("p (a b) -> p a b", a=8),
                    xg_b[i, :, NSH:NSH + 8].rearrange("(p a) b -> p a b",
                                                      p=128))
            a_s = wts.tile([128, CCH, 64], F32R)
            nc.gpsimd.tensor_copy(a_s[:], a_h[:])
            bB_h = wts.tile([42, 256], F16)
            nc.sync.dma_start(bB_h[:], bB.ap())
            bB_s = wts.tile([42, 256], F32R)
            nc.gpsimd.tensor_copy(bB_s[:], bB_h[:])
            bq_s = wts.tile([128, 1], F32)
            nc.sync.dma_start(bq_s[:], bq.ap())
            bv_s = wts.tile([128, 1], F32)
            nc.sync.dma_start(bv_s[:], bv.ap())
            wo_h = wts.tile([128, CCH, 128], F16)
            nc.sync.dma_start(wo_h[:], woT.ap().rearrange("i p m -> p i m"))
            wo_s = wts.tile([128, CCH, 128], F32R)
            nc.gpsimd.tensor_copy(wo_s[:], wo_h[:])
            mb_s = wts.tile([128, B * KCH], F32)
            nc.sync.dma_start(mb_s[:], mb.ap())
            # identity for PE transposes, built on-chip: free_idx - part_idx == 0
            io32 = wts.tile([128, 128], mybir.dt.int32)
            nc.gpsimd.iota(io32[:], pattern=[[1, 128]], base=0,
                           channel_multiplier=-1)
            ident = wts.tile([128, 128], F32R)
            nc.gpsimd.tensor_scalar(ident[:], io32[:], 0, None,
                                    mybir.AluOpType.is_equal)
            ones_s = wts.tile([128, KCH], F32R)
            nc.gpsimd.tensor_scalar(ones_s[:], io32[:, 0:KCH], -(1 << 30),
                                    None, mybir.AluOpType.is_gt)

            # --- persistent activations ---
            qT = acts.tile([128, BN], F32R)
            kT = acts.tile([128, BN], F32R)
            vT = acts.tile([128, BN], F32R)
            aoT = acts.tile([128, BN], F32R)

            # ---------- phase 1: projections ----------
            for nch in range(NCH):
                nsl = bass.ts(nch, 512)
                x_h = xhp.tile([128, CCH, 512], F16)
                nc.sync.dma_start(
                    x_h[:],
                    xg_b[nch, :, 0:NSH].rearrange("(i p) m -> p i m", p=128))
                x_t = xin.tile([128, CCH, 512], F32R)
                nc.gpsimd.tensor_copy(x_t[:], x_h[:])

                z_ps = ps_o.tile([64, 512], F32, tag="o")
                for i in range(CCH):
                    nc.tensor.matmul(z_ps[:], (a_s[:, i, :]), (x_t[:, i, :]),
                                     start=(i == 0), stop=(i == CCH - 1))
                z_t = ztp.tile([64, 512], F32R)
                nc.vector.tensor_copy(z_t[:], z_ps[:])

                q_ps = ps_s.tile([128, 512], F32, tag="s")
                for i in range(CCH):
                    nc.tensor.matmul(q_ps[:], (wq_s[:, i, :]), (x_t[:, i, :]),
                                     start=(i == 0), stop=False)
                nc.tensor.matmul(q_ps[:], (bB_s[0:R, 0:128]), (z_t[0:R, :]),
                                 start=False, stop=True)
                nc.scalar.activation(qT[:, nsl], q_ps[:],
                                     mybir.ActivationFunctionType.Identity,
                                     bias=bq_s[:])

                k_ps = ps_s.tile([128, 512], F32, tag="s")
                for i in range(CCH):
                    nc.tensor.matmul(k_ps[:], (wk_s[:, i, :]), (x_t[:, i, :]),
                                     start=(i == 0), stop=(i == CCH - 1))
                nc.vector.tensor_copy(kT[:, nsl], k_ps[:])

                v_ps = ps_s.tile([128, 512], F32, tag="s")
                for i in range(CCH):
                    nc.tensor.matmul(v_ps[:], (wv_s[:, i, :]), (x_t[:, i, :]),
                                     start=(i == 0), stop=False)
                nc.tensor.matmul(v_ps[:], (bB_s[32:32 + R, 128:256]),
                                 (z_t[32:32 + R, :]), start=False, stop=True)
                nc.scalar.activation(vT[:, nsl], v_ps[:],
                                     mybir.ActivationFunctionType.Identity,
                                     bias=bv_s[:])

            # ---------- phase 2: attention ----------
            for b in range(B):
                for hh in range(2):
                    hsl = bass.ds(hh * 64, 64)
                    kb = b * N
                    v_sb = vsbp.tile([128, KCH, 65], F32R)
                    nc.vector.tensor_copy(v_sb[:, :, 64:65], ones_s[:])
                    for g in range(2):
                        vt_ps = ps_s.tile([128, 8, 64], F32R, tag="s")
                        for j in range(8):
                            kc = g * 8 + j
                            nc.tensor.transpose(
                                vt_ps[:, j, :],
                                vT[hsl, bass.ds(kb + kc * 128, 128)],
                                ident[hsl, hsl])
                        nc.vector.tensor_copy(
                            v_sb[:, g * 8:(g + 1) * 8, 0:64], vt_ps[:])

                    for qc in range(QCH):
                        qsl = bass.ds(kb + qc * 512, 512)
                        q_ap = qT[hsl, qsl]
                        o_ps = ps_o.tile([65, 512], F32, tag="o")
                        for g in range(KCH // 2):
                            s_ps = ps_s2.tile([128, 2, 512], F32, tag="s2")
                            for j in range(2):
                                kc = g * 2 + j
                                nc.tensor.matmul(
                                    s_ps[:, j, :],
                                    (kT[hsl, bass.ds(kb + kc * 128, 128)]),
                                    (q_ap), start=True, stop=True)
                            p_sb = ptp.tile([128, 2, 512], F32R)
                            nc.scalar.activation(
                                p_sb[:], s_ps[:],
                                mybir.ActivationFunctionType.Exp,
                                bias=mb_s[:, bass.ds(b * KCH + g * 2, 1)],
                                scale=ATT_SCALE)
                            for j in range(2):
                                kc = g * 2 + j
                                nc.tensor.matmul(o_ps[:], (v_sb[:, kc, :]),
                                                 (p_sb[:, j, :]),
                                                 start=(kc == 0),
                                                 stop=(kc == KCH - 1))
                        rec = recp.tile([1, 512], F32)
                        nc.vector.reciprocal(rec[:], o_ps[64:65, :])
                        rbc = rbcp.tile([64, 512], F32)
                        nc.gpsimd.partition_broadcast(rbc[:], rec[:])
                        nc.vector.tensor_mul(aoT[hsl, qsl], o_ps[0:64, :], rbc[:])

            if dbg:
                nc.sync.dma_start(dbg_q.ap(), qT[:].bitcast(F32))
                nc.sync.dma_start(dbg_k.ap(), kT[:].bitcast(F32))
                nc.sync.dma_start(dbg_v.ap(), vT[:].bitcast(F32))
                nc.sync.dma_start(dbg_ao.ap(), aoT[:].bitcast(F32))

            # ---------- phase 3: output projection (partial, per core) ----
            for nch in range(NCH):
                nsl = bass.ts(nch, 512)
                for ci in range(CCH):
                    y_ps = ps_s.tile([128, 512], F32, tag="s")
                    nc.tensor.matmul(y_ps[:], (wo_s[:, ci, :]), (aoT[:, nsl]),
                                     start=True, stop=True)
                    y_sb = youtp.tile([128, 512], F32)
                    if ci % 2 == 0:
                        nc.scalar.copy(y_sb[:], y_ps[:])
                    else:
                        nc.vector.tensor_copy(y_sb[:], y_ps[:])
                    nc.sync.dma_start(y_b[ci, :, nsl], y_sb[:])

            # reduce the partials across cores; core s keeps C-rows slice s
            nc.gpsimd.collective_compute(
                "ReduceScatter", mybir.AluOpType.add,
                replica_groups=[list(range(NCORES))],
                ins=[y_b.opt()], outs=[yr_b.opt()])
            # int8-quantize the final slice (per-partition scale) for the wire
            y_f = ycv.tile([128, BN], F32)
            nc.sync.dma_start(y_f[:], yr_b[:])
            amax = ycv.tile([128, 1], F32)
            nc.vector.tensor_reduce(amax[:], y_f[:], mybir.AxisListType.X,
                                    mybir.AluOpType.max,
                                    apply_absolute_value=True)
            nc.vector.tensor_scalar_max(amax[:], amax[:], 1e-20)
            nc.sync.dma_start(ysc.ap(), amax[:])
            rcp = ycv.tile([128, 1], F32)
            nc.vector.reciprocal(rcp[:], amax[:])
            qsc = ycv.tile([128, 1], F32)
            nc.vector.tensor_scalar_mul(qsc[:], rcp[:], 127.0)
            yq = ycv.tile([128, BN], mybir.dt.int8)
            nc.vector.tensor_scalar_mul(yq[:], y_f[:], qsc[:])
            nc.sync.dma_start(yq8.ap(), yq[:])
    nc.compile()
    return nc


_NC = None


def _get_nc():
    global _NC
    if _NC is None:
        _NC = build_nc()
    return _NC


def _bB(Bq_sl, Bv_sl):
    out = np.zeros((42, 256), np.float16)
    out[0:R, 0:128] = (Bq_sl * SCALING).T
    out[32:32 + R, 128:256] = (Bv_sl * SCALING).T
    return out


def _prep_in_maps(inputs):
    x = np.asarray(inputs["x"], np.float32)
    mask = np.asarray(inputs["mask"])
    W_qkv = np.asarray(inputs["W_qkv"], np.float32)
    Wq_base = np.asarray(inputs["Wq_base"], np.float32)
    bq = np.asarray(inputs["bq"], np.float32)
    Aq = np.asarray(inputs["Aq"], np.float32)
    Bq = np.asarray(inputs["Bq"], np.float32)
    Wv_base = np.asarray(inputs["Wv_base"], np.float32)
    bv = np.asarray(inputs["bv"], np.float32)
    Av = np.asarray(inputs["Av"], np.float32)
    Bv = np.asarray(inputs["Bv"], np.float32)
    W_out = np.asarray(inputs["W_out"], np.float32)

    xT = np.ascontiguousarray(x.reshape(BN, C).T)
    Wq_eff = W_qkv[0:H * D] + Wq_base
    Wk = W_qkv[H * D:2 * H * D]
    Wv_eff = W_qkv[2 * H * D:3 * H * D] + Wv_base
    aT = np.zeros((C, 64), np.float32)
    aT[:, 0:R] = Aq.T
    aT[:, 32:32 + R] = Av.T
    mbias = np.where(mask.reshape(BN), 0.0, -1e5).astype(np.float32)
    mb = np.ascontiguousarray(mbias.reshape(B * KCH, 128).T)

    in_maps = []
    for s in range(NCORES):
        sl = slice(s * 128, (s + 1) * 128)
        xa = np.concatenate(
            [xT[:, s * NSH:(s + 1) * NSH], aT[sl].reshape(C, 8)], axis=1)
        in_maps.append({
            "xTs": np.ascontiguousarray(xa).astype(np.float16),
            "wqT": np.ascontiguousarray(Wq_eff[sl].T).astype(np.float16),
            "wkT": np.ascontiguousarray(Wk[sl].T).astype(np.float16),
            "wvT": np.ascontiguousarray(Wv_eff[sl].T).astype(np.float16),
            "bB": _bB(Bq[sl], Bv[sl]),
            "bq": np.ascontiguousarray(bq[sl, None]),
            "bv": np.ascontiguousarray(bv[sl, None]),
            "woT": np.ascontiguousarray(
                W_out[:, sl].reshape(CCH, 128, 128).transpose(0, 2, 1)
            ).astype(np.float16),
            "mb": mb,
        })
    return in_maps


def _assemble(results, b_out):
    yT = np.concatenate(
        [r["yq8"].astype(np.float32) * (r["ysc"] / 127.0) for r in results],
        axis=0)  # [C, BN]
    out = yT.T + np.asarray(b_out, np.float32)[None, :]
    return np.ascontiguousarray(out.reshape(B, N, C).astype(np.float32))


def kernel(**inputs):
    nc = _get_nc()
    in_maps = _prep_in_maps(inputs)
    res = run_bass_kernel_spmd(nc, in_maps, core_ids=list(range(NCORES)))
    return _assemble(res.results, inputs["b_out"])


def run_traced(inputs):
    """test harness hook: returns (output, exec_time_ns)."""
    nc = _get_nc()
    in_maps = _prep_in_maps(inputs)
    res = run_bass_kernel_spmd(nc, in_maps, core_ids=list(range(NCORES)),
                               trace=True)
    return _assemble(res.results, inputs["b_out"]), res.exec_time_ns


# revision 43
# speedup vs baseline: 16.9256x; 1.1230x over previous
"""LoRA attention kernel for 8 trn2 NeuronCores, tensor-parallel over heads.

Sharding: core s owns heads 2s, 2s+1 (a 128-row slice of the HD=1024 dim).
Host->device traffic is minimized (the axon tunnel moves ~45MB/s, so wire
bytes dominate the spmd-call wall time):
  - x is shipped token-sharded (each core gets 512 of the 4096 tokens,
    transposed to [C, 512]) and AllGathered on-device over NeuronLink.
  - x and the large weights travel the wire as packed 12-bit ints
    (int8 hi byte + packed lo nibbles, one global scale per tensor; adds
    ~4e-3 rel err vs the 2e-2 budget) and are unpacked to f32r on-device so
    all matmul numerics match the f32 version. The per-core slice of the
    LoRA A matrix rides the same AllGather as x (12 extra packed columns).
  - each core computes q/k/v projections (base + LoRA fused), attention for
    its 4 (batch, head) pairs, and a partial output projection [C, BN];
    the partials are ReduceScattered (f32) on-device so each core returns
    only its 128-row slice of the final y^T, int8-quantized with a
    per-row scale (adds ~3e-3 rel err). The host dequantizes, stacks the 8
    slices, and adds b_out.

Layouts (per core, on-chip):
  xT   [C=1024, B*N=4096]   activations transposed (contraction dim C on
                            partitions, 8 chunks of 128)
  qT/kT/vT [128, 4096]      2 heads x 64 dims on partitions
  attention runs in S^T layout: S^T[k, q] = K^T.T @ Q^T per 128-key chunk,
  exp via ScalarE (mask folded in as a per-partition additive bias), then
  O^T accumulated with lhsT = [V | ones] so the softmax denominator falls
  out of the same matmuls as PSUM row 64.
"""

import numpy as np

import jax

try:
    # Each run_bass_kernel_spmd call re-jits (fresh closures inside the
    # library), so the persistent compile cache saves ~80ms/call.
    jax.config.update("jax_compilation_cache_dir", "/tmp/jax_comp_cache")
    jax.config.update("jax_persistent_cache_min_entry_size_bytes", 0)
    jax.config.update("jax_persistent_cache_min_compile_time_secs", 0.0)
except Exception:
    pass

import concourse.bass as bass
import concourse.tile as tile
from concourse import bacc, mybir
from concourse.bass_utils import run_bass_kernel_spmd

H, D, R, C, B, N = 16, 64, 10, 1024, 2, 2048
BN = B * N
SCALING = 1.0 / R
ATT_SCALE = float(D) ** -0.5
NCORES = 8
F32 = mybir.dt.float32
F32R = mybir.dt.float32r
F16 = mybir.dt.float16
U8 = mybir.dt.uint8
I8 = mybir.dt.int8
NCH = BN // 512  # 8 n-chunks of 512
CCH = C // 128  # 8 contraction chunks
KCH = N // 128  # 16 key chunks per (b,h)
QCH = N // 512  # 4 query chunks per (b,h)
NSH = BN // NCORES  # 512 tokens per core shard


def _unpack12(nc, dst, hi, lo, cols, sc, pool, tag):
    """dst[f32*] = (int8(hi)*16 + nibbles(lo)) * sc.

    hi [128, cols] uint8 (int8 bit pattern), lo [128, cols//2] uint8
    (hi nibble = even element, lo nibble = odd).
    """
    ev = pool.tile([128, cols // 2], U8, tag=tag + "e")
    nc.vector.tensor_scalar(ev[:], lo, 4, None,
                            mybir.AluOpType.logical_shift_right)
    od = pool.tile([128, cols // 2], U8, tag=tag + "o")
    nc.vector.tensor_scalar(od[:], lo, 15, None,
                            mybir.AluOpType.bitwise_and)
    lo_f = pool.tile([128, cols], F32, tag=tag + "l")
    lo_v = lo_f[:].rearrange("p (u two) -> p u two", two=2)
    nc.vector.tensor_copy(lo_v[:, :, 0], ev[:])
    nc.vector.tensor_copy(lo_v[:, :, 1], od[:])
    hi_f = pool.tile([128, cols], F32, tag=tag + "h")
    nc.vector.tensor_copy(hi_f[:], hi.bitcast(I8))
    cmb = pool.tile([128, cols], F32, tag=tag + "c")
    nc.vector.scalar_tensor_tensor(
        cmb[:], hi_f[:], 16.0, lo_f[:],
        mybir.AluOpType.mult, mybir.AluOpType.add)
    nc.vector.tensor_scalar_mul(dst, cmb[:], sc)


def build_nc(dbg=False):
    nc = bacc.Bacc("TRN2", target_bir_lowering=False, debug=False,
                   num_devices=NCORES)
    if dbg:
        dbg_q = nc.dram_tensor("dbg_q", [128, BN], F32, kind="ExternalOutput")
        dbg_k = nc.dram_tensor("dbg_k", [128, BN], F32, kind="ExternalOutput")
        dbg_v = nc.dram_tensor("dbg_v", [128, BN], F32, kind="ExternalOutput")
        dbg_ao = nc.dram_tensor("dbg_ao", [128, BN], F32, kind="ExternalOutput")
    # x12 carries, int12-packed per column group: this core's 512-token
    # slice of x^T (hi bytes 0:512, lo nibble-pairs 520:776) plus its
    # 128-row slice of aT packed [128,64]->[1024,8] (hi 512:520, lo
    # 776:780), so aT rides the same AllGather as x.
    x12 = nc.dram_tensor("x12", [C, 780], U8, kind="ExternalInput")
    w12q = nc.dram_tensor("w12q", [C, 192], U8, kind="ExternalInput")
    w12k = nc.dram_tensor("w12k", [C, 192], U8, kind="ExternalInput")
    w12v = nc.dram_tensor("w12v", [C, 192], U8, kind="ExternalInput")
    wo12 = nc.dram_tensor("wo12", [CCH, 128, 192], U8, kind="ExternalInput")
    scs = nc.dram_tensor("scs", [128, 8], F32, kind="ExternalInput")
    bB = nc.dram_tensor("bB", [42, 256], F16, kind="ExternalInput")
    bq = nc.dram_tensor("bq", [128, 1], F32, kind="ExternalInput")
    bv = nc.dram_tensor("bv", [128, 1], F32, kind="ExternalInput")
    mb = nc.dram_tensor("mb", [128, B * KCH], F32, kind="ExternalInput")
    yq8 = nc.dram_tensor("yq8", [128, BN], mybir.dt.int8, kind="ExternalOutput")
    ysc = nc.dram_tensor("ysc", [128, 1], F32, kind="ExternalOutput")

    from contextlib import ExitStack
    with tile.TileContext(nc) as tc:
        with ExitStack() as st:
            pool = lambda **kw: st.enter_context(tc.tile_pool(**kw))
            dram = pool(name="dram", bufs=1, space="DRAM")
            wts = pool(name="wts", bufs=1)
            acts = pool(name="acts", bufs=1)
            xin = pool(name="xin", bufs=1)
            xhp = pool(name="xhp", bufs=2)
            upk = pool(name="upk", bufs=2)
            ycv = pool(name="ycv", bufs=1)
            ztp = pool(name="zt", bufs=2)
            ptp = pool(name="pt", bufs=4)
            vsbp = pool(name="vsb", bufs=2)
            recp = pool(name="rec", bufs=2)
            rbcp = pool(name="rbc", bufs=2)
            youtp = pool(name="yout", bufs=4)
            ps_s = pool(name="ps_s", bufs=2, space="PSUM")
            ps_s2 = pool(name="ps_s2", bufs=2, space="PSUM")
            ps_o = pool(name="ps_o", bufs=2, space="PSUM")
            # --- DRAM bounce buffers for collectives ---
            xs_b = dram.tile([C, 780], U8)
            xg_b = dram.tile([NCH, C, 780], U8, addr_space="Shared")
            y_b = dram.tile([CCH, 128, BN], F32)
            yr_b = dram.tile([128, BN], F32)

            # gather the full xT across cores: core s contributes tokens
            # [512s, 512s+512), so gathered chunk nch = token chunk nch.
            nc.sync.dma_start(xs_b[:], x12.ap())
            nc.gpsimd.collective_compute(
                "AllGather", mybir.AluOpType.bypass,
                replica_groups=[list(range(NCORES))],
                ins=[xs_b.opt()], outs=[xg_b.opt()])

            scs_s = wts.tile([128, 8], F32)
            nc.sync.dma_start(scs_s[:], scs.ap())

            # --- resident weights (wire int12, unpack to f32r on-chip) ---
            wq_u = wts.tile([128, CCH, 192], U8)
            nc.sync.dma_start(wq_u[:], w12q.ap().rearrange("(i p) m -> p i m", p=128))
            wk_u = wts.tile([128, CCH, 192], U8)
            nc.sync.dma_start(wk_u[:], w12k.ap().rearrange("(i p) m -> p i m", p=128))
            wv_u = wts.tile([128, CCH, 192], U8)
            nc.sync.dma_start(wv_u[:], w12v.ap().rearrange("(i p) m -> p i m", p=128))
            wo_u = wts.tile([128, CCH, 192], U8)
            nc.sync.dma_start(wo_u[:], wo12.ap().rearrange("i p m -> p i m"))
            wq_s = wts.tile([128, CCH, 128], F32R)
            wk_s = wts.tile([128, CCH, 128], F32R)
            wv_s = wts.tile([128, CCH, 128], F32R)
            wo_s = wts.tile([128, CCH, 128], F32R)
            for w_u, w_s, ci in ((wq_u, wq_s, 2), (wk_u, wk_s, 3),
                                 (wv_u, wv_s, 4), (wo_u, wo_s, 5)):
                for i in range(CCH):
                    _unpack12(nc, w_s[:, i, :], w_u[:, i, 0:128],
                             w_u[:, i, 128:192], 128, scs_s[:, ci:ci + 1],
                             upk, "wu")

            # aT arrives inside the gathered x buffer: chunk i's columns
            # 512:520 (hi) and 776:780 (lo) unpack to aT rows [128i, 128i+128).
            a_hu = wts.tile([128, CCH * 64], U8)
            a_lu = wts.tile([128, CCH * 32], U8)
            for i in range(CCH):
                nc.sync.dma_start(
                    a_hu[:, i * 64:(i + 1) * 64].rearrange(
                        "p (a b) -> p a b", a=8),
                    xg_b[i, :, 512:520].rearrange("(p a) b -> p a b", p=128))
                nc.sync.dma_start(
                    a_lu[:, i * 32:(i + 1) * 32].rearrange(
                        "p (a j) -> p a j", a=8),
                    xg_b[i, :, 776:780].rearrange("(p a) j -> p a j", p=128))
            a_s = wts.tile([128, CCH, 64], F32R)
            _unpack12(nc, a_s[:].rearrange("p i m -> p (i m)"), a_hu[:], a_lu[:],
                     CCH * 64, scs_s[:, 1:2], upk, "xu")
            bB_h = wts.tile([42, 256], F16)
            nc.sync.dma_start(bB_h[:], bB.ap())
            bB_s = wts.tile([42, 256], F32R)
            nc.gpsimd.tensor_copy(bB_s[:], bB_h[:])
            bq_s = wts.tile([128, 1], F32)
            nc.sync.dma_start(bq_s[:], bq.ap())
            bv_s = wts.tile([128, 1], F32)
            nc.sync.dma_start(bv_s[:], bv.ap())
            mb_s = wts.tile([128, B * KCH], F32)
            nc.sync.dma_start(mb_s[:], mb.ap())
            # identity for PE transposes, built on-chip: free_idx - part_idx == 0
            io32 = wts.tile([128, 128], mybir.dt.int32)
            nc.gpsimd.iota(io32[:], pattern=[[1, 128]], base=0,
                           channel_multiplier=-1)
            ident = wts.tile([128, 128], F32R)
            nc.gpsimd.tensor_scalar(ident[:], io32[:], 0, None,
                                    mybir.AluOpType.is_equal)
            ones_s = wts.tile([128, KCH], F32R)
            nc.gpsimd.tensor_scalar(ones_s[:], io32[:, 0:KCH], -(1 << 30),
                                    None, mybir.AluOpType.is_gt)

            # --- persistent activations ---
            qT = acts.tile([128, BN], F32R)
            kT = acts.tile([128, BN], F32R)
            vT = acts.tile([128, BN], F32R)
            aoT = acts.tile([128, BN], F32R)

            # ---------- phase 1: projections ----------
            for nch in range(NCH):
                nsl = bass.ts(nch, 512)
                xh8 = xhp.tile([128, CCH, 512], U8)
                nc.sync.dma_start(
                    xh8[:],
                    xg_b[nch, :, 0:512].rearrange("(i p) m -> p i m", p=128))
                xl8 = xhp.tile([128, CCH, 256], U8)
                nc.sync.dma_start(
                    xl8[:],
                    xg_b[nch, :, 520:776].rearrange("(i p) m -> p i m", p=128))
                x_t = xin.tile([128, CCH, 512], F32R)
                for i in range(CCH):
                    _unpack12(nc, x_t[:, i, :], xh8[:, i, :], xl8[:, i, :], 512,
                             scs_s[:, 0:1], upk, "xu")

                z_ps = ps_o.tile([64, 512], F32, tag="o")
                for i in range(CCH):
                    nc.tensor.matmul(z_ps[:], (a_s[:, i, :]), (x_t[:, i, :]),
                                     start=(i == 0), stop=(i == CCH - 1))
                z_t = ztp.tile([64, 512], F32R)
                nc.vector.tensor_copy(z_t[:], z_ps[:])

                q_ps = ps_s.tile([128, 512], F32, tag="s")
                for i in range(CCH):
                    nc.tensor.matmul(q_ps[:], (wq_s[:, i, :]), (x_t[:, i, :]),
                                     start=(i == 0), stop=False)
                nc.tensor.matmul(q_ps[:], (bB_s[0:R, 0:128]), (z_t[0:R, :]),
                                 start=False, stop=True)
                nc.scalar.activation(qT[:, nsl], q_ps[:],
                                     mybir.ActivationFunctionType.Identity,
                                     bias=bq_s[:])

                k_ps = ps_s.tile([128, 512], F32, tag="s")
                for i in range(CCH):
                    nc.tensor.matmul(k_ps[:], (wk_s[:, i, :]), (x_t[:, i, :]),
                                     start=(i == 0), stop=(i == CCH - 1))
                nc.vector.tensor_copy(kT[:, nsl], k_ps[:])

                v_ps = ps_s.tile([128, 512], F32, tag="s")
                for i in range(CCH):
                    nc.tensor.matmul(v_ps[:], (wv_s[:, i, :]), (x_t[:, i, :]),
                                     start=(i == 0), stop=False)
                nc.tensor.matmul(v_ps[:], (bB_s[32:32 + R, 128:256]),
                                 (z_t[32:32 + R, :]), start=False, stop=True)
                nc.scalar.activation(vT[:, nsl], v_ps[:],
                                     mybir.ActivationFunctionType.Identity,
                                     bias=bv_s[:])

            # ---------- phase 2: attention ----------
            for b in range(B):
                for hh in range(2):
                    hsl = bass.ds(hh * 64, 64)
                    kb = b * N
                    v_sb = vsbp.tile([128, KCH, 65], F32R)
                    nc.vector.tensor_copy(v_sb[:, :, 64:65], ones_s[:])
                    for g in range(2):
                        vt_ps = ps_s.tile([128, 8, 64], F32R, tag="s")
                        for j in range(8):
                            kc = g * 8 + j
                            nc.tensor.transpose(
                                vt_ps[:, j, :],
                                vT[hsl, bass.ds(kb + kc * 128, 128)],
                                ident[hsl, hsl])
                        nc.vector.tensor_copy(
                            v_sb[:, g * 8:(g + 1) * 8, 0:64], vt_ps[:])

                    for qc in range(QCH):
                        qsl = bass.ds(kb + qc * 512, 512)
                        q_ap = qT[hsl, qsl]
                        o_ps = ps_o.tile([65, 512], F32, tag="o")
                        for g in range(KCH // 2):
                            s_ps = ps_s2.tile([128, 2, 512], F32, tag="s2")
                            for j in range(2):
                                kc = g * 2 + j
                                nc.tensor.matmul(
                                    s_ps[:, j, :],
                                    (kT[hsl, bass.ds(kb + kc * 128, 128)]),
                                    (q_ap), start=True, stop=True)
                            p_sb = ptp.tile([128, 2, 512], F32R)
                            nc.scalar.activation(
                                p_sb[:], s_ps[:],
                                mybir.ActivationFunctionType.Exp,
                                bias=mb_s[:, bass.ds(b * KCH + g * 2, 1)],
                                scale=ATT_SCALE)
                            for j in range(2):
                                kc = g * 2 + j
                                nc.tensor.matmul(o_ps[:], (v_sb[:, kc, :]),
                                                 (p_sb[:, j, :]),
                                                 start=(kc == 0),
                                                 stop=(kc == KCH - 1))
                        rec = recp.tile([1, 512], F32)
                        nc.vector.reciprocal(rec[:], o_ps[64:65, :])
                        rbc = rbcp.tile([64, 512], F32)
                        nc.gpsimd.partition_broadcast(rbc[:], rec[:])
                        nc.vector.tensor_mul(aoT[hsl, qsl], o_ps[0:64, :], rbc[:])

            if dbg:
                nc.sync.dma_start(dbg_q.ap(), qT[:].bitcast(F32))
                nc.sync.dma_start(dbg_k.ap(), kT[:].bitcast(F32))
                nc.sync.dma_start(dbg_v.ap(), vT[:].bitcast(F32))
                nc.sync.dma_start(dbg_ao.ap(), aoT[:].bitcast(F32))

            # ---------- phase 3: output projection (partial, per core) ----
            for nch in range(NCH):
                nsl = bass.ts(nch, 512)
                for ci in range(CCH):
                    y_ps = ps_s.tile([128, 512], F32, tag="s")
                    nc.tensor.matmul(y_ps[:], (wo_s[:, ci, :]), (aoT[:, nsl]),
                                     start=True, stop=True)
                    y_sb = youtp.tile([128, 512], F32)
                    if ci % 2 == 0:
                        nc.scalar.copy(y_sb[:], y_ps[:])
                    else:
                        nc.vector.tensor_copy(y_sb[:], y_ps[:])
                    nc.sync.dma_start(y_b[ci, :, nsl], y_sb[:])

            # reduce the partials across cores; core s keeps C-rows slice s
            nc.gpsimd.collective_compute(
                "ReduceScatter", mybir.AluOpType.add,
                replica_groups=[list(range(NCORES))],
                ins=[y_b.opt()], outs=[yr_b.opt()])
            # int8-quantize the final slice (per-partition scale) for the wire
            y_f = ycv.tile([128, BN], F32)
            nc.sync.dma_start(y_f[:], yr_b[:])
            amax = ycv.tile([128, 1], F32)
            nc.vector.tensor_reduce(amax[:], y_f[:], mybir.AxisListType.X,
                                    mybir.AluOpType.max,
                                    apply_absolute_value=True)
            nc.vector.tensor_scalar_max(amax[:], amax[:], 1e-20)
            nc.sync.dma_start(ysc.ap(), amax[:])
            rcp = ycv.tile([128, 1], F32)
            nc.vector.reciprocal(rcp[:], amax[:])
            qsc = ycv.tile([128, 1], F32)
            nc.vector.tensor_scalar_mul(qsc[:], rcp[:], 127.0)
            yq = ycv.tile([128, BN], mybir.dt.int8)
            nc.vector.tensor_scalar_mul(yq[:], y_f[:], qsc[:])
            nc.sync.dma_start(yq8.ap(), yq[:])
    nc.compile()
    return nc


_NC = None


def _get_nc():
    global _NC
    if _NC is None:
        _NC = build_nc()
    return _NC


def _bB(Bq_sl, Bv_sl):
    out = np.zeros((42, 256), np.float16)
    out[0:R, 0:128] = (Bq_sl * SCALING).T
    out[32:32 + R, 128:256] = (Bv_sl * SCALING).T
    return out


def _pack12(a):
    """int12 quantize with one global scale: int8 hi bytes + packed nibbles."""
    a = np.ascontiguousarray(a, np.float32)
    s = max(float(np.abs(a).max()) / 2047.0, 1e-30)
    v = np.clip(np.round(a / s), -2047, 2047).astype(np.int32)
    hi = (v >> 4).astype(np.int8).view(np.uint8)
    lo = (v & 15).astype(np.uint8)
    lo2 = ((lo[..., 0::2] << 4) | lo[..., 1::2]).astype(np.uint8)
    return hi, lo2, np.float32(s)


def _prep_in_maps(inputs):
    x = np.asarray(inputs["x"], np.float32)
    mask = np.asarray(inputs["mask"])
    W_qkv = np.asarray(inputs["W_qkv"], np.float32)
    Wq_base = np.asarray(inputs["Wq_base"], np.float32)
    bq = np.asarray(inputs["bq"], np.float32)
    Aq = np.asarray(inputs["Aq"], np.float32)
    Bq = np.asarray(inputs["Bq"], np.float32)
    Wv_base = np.asarray(inputs["Wv_base"], np.float32)
    bv = np.asarray(inputs["bv"], np.float32)
    Av = np.asarray(inputs["Av"], np.float32)
    Bv = np.asarray(inputs["Bv"], np.float32)
    W_out = np.asarray(inputs["W_out"], np.float32)

    xT = np.ascontiguousarray(x.reshape(BN, C).T)
    Wq_eff = W_qkv[0:H * D] + Wq_base
    Wk = W_qkv[H * D:2 * H * D]
    Wv_eff = W_qkv[2 * H * D:3 * H * D] + Wv_base
    aT = np.zeros((C, 64), np.float32)
    aT[:, 0:R] = Aq.T
    aT[:, 32:32 + R] = Av.T
    mbias = np.where(mask.reshape(BN), 0.0, -1e5).astype(np.float32)
    mb = np.ascontiguousarray(mbias.reshape(B * KCH, 128).T)

    xhi, xlo, s_x = _pack12(xT)          # [C, BN], [C, BN//2]
    ahi, alo, s_a = _pack12(aT)          # [C, 64], [C, 32]

    in_maps = []
    for s in range(NCORES):
        sl = slice(s * 128, (s + 1) * 128)
        x12 = np.concatenate(
            [xhi[:, s * NSH:(s + 1) * NSH], ahi[sl].reshape(C, 8),
             xlo[:, s * (NSH // 2):(s + 1) * (NSH // 2)],
             alo[sl].reshape(C, 4)], axis=1)
        qhi, qlo, s_wq = _pack12(np.ascontiguousarray(Wq_eff[sl].T))
        khi, klo, s_wk = _pack12(np.ascontiguousarray(Wk[sl].T))
        vhi, vlo, s_wv = _pack12(np.ascontiguousarray(Wv_eff[sl].T))
        ohi, olo, s_wo = _pack12(np.ascontiguousarray(
            W_out[:, sl].reshape(CCH, 128, 128).transpose(0, 2, 1)))
        scs = np.zeros((128, 8), np.float32)
        scs[:, 0], scs[:, 1] = s_x, s_a
        scs[:, 2], scs[:, 3], scs[:, 4], scs[:, 5] = s_wq, s_wk, s_wv, s_wo
        in_maps.append({
            "x12": np.ascontiguousarray(x12),
            "w12q": np.ascontiguousarray(np.concatenate([qhi, qlo], axis=1)),
            "w12k": np.ascontiguousarray(np.concatenate([khi, klo], axis=1)),
            "w12v": np.ascontiguousarray(np.concatenate([vhi, vlo], axis=1)),
            "wo12": np.ascontiguousarray(np.concatenate([ohi, olo], axis=2)),
            "scs": scs,
            "bB": _bB(Bq[sl], Bv[sl]),
            "bq": np.ascontiguousarray(bq[sl, None]),
            "bv": np.ascontiguousarray(bv[sl, None]),
            "mb": mb,
        })
    return in_maps


def _assemble(results, b_out):
    yT = np.concatenate(
        [r["yq8"].astype(np.float32) * (r["ysc"] / 127.0) for r in results],
        axis=0)  # [C, BN]
    out = yT.T + np.asarray(b_out, np.float32)[None, :]
    return np.ascontiguousarray(out.reshape(B, N, C).astype(np.float32))


def kernel(**inputs):
    nc = _get_nc()
    in_maps = _prep_in_maps(inputs)
    res = run_bass_kernel_spmd(nc, in_maps, core_ids=list(range(NCORES)))
    return _assemble(res.results, inputs["b_out"])


def run_traced(inputs):
    """test harness hook: returns (output, exec_time_ns)."""
    nc = _get_nc()
    in_maps = _prep_in_maps(inputs)
    res = run_bass_kernel_spmd(nc, in_maps, core_ids=list(range(NCORES)),
                               trace=True)
    return _assemble(res.results, inputs["b_out"]), res.exec_time_ns
